# revision 1
# baseline (speedup 1.0000x reference)
"""GTrXL layer (TransformerXL attention + GRU gating) on 8 TRN2 NeuronCores.

Sharding: pure data-parallel over batch (BS=8 -> 1 batch element per core).
No collectives. Per-core Bass/Tile kernel computes the full layer for its
batch element.

Layout convention on-chip: activations are kept TRANSPOSED [feature, token]
(feature on partitions, 128-chunks) so that weight matrices in natural [K, N]
layout serve as the stationary matmul operand and matmul outputs land
transposed again:  outT[n, t] = sum_k W[k, n] * xT[k, t].

Matmul compute in bf16 (f32 accumulate in PSUM); LN/softmax/GRU elementwise
math in f32.

Relative-shift: pos scores P[i, relk] are written per 128-query-chunk to a
DRAM scratch of row stride 1536 whose tail 512 columns are pre-filled with
-1e30; the shifted read  shifted[i, j] = P[i, 511 + j - i]  is a single
strided DMA (row step 1535), and the pad lands exactly on the masked region
j > i + 512, so masking comes for free.
"""

import sys

if '/opt/trn_rl_repo' not in sys.path:
    sys.path.insert(0, '/opt/trn_rl_repo')

import numpy as np
import ml_dtypes

import concourse.bass as bass
import concourse.tile as tile
from concourse import bacc, mybir
from concourse.bass_utils import run_bass_kernel_spmd
from concourse.masks import make_identity

BF16 = mybir.dt.bfloat16
F32 = mybir.dt.float32

HEAD_NUM, HEAD_DIM = 16, 64
D, HID = 1024, 4096
CUR, PREV, BS = 512, 512, 8
FULL = CUR + PREV
EPS = 1e-5
SCALE = 1.0 / (HEAD_DIM ** 0.5)
P = 128
DC = D // P          # 8 feature chunks
HC = HID // P        # 32 hidden chunks
TCF = FULL // P      # 8 full-token chunks
TCC = CUR // P       # 4 query-token chunks
NEG = -1.0e30

AluOp = mybir.AluOpType
Act = mybir.ActivationFunctionType


def _dram_in(dram, name, shape, dtype):
    return dram.tile(list(shape), dtype, kind="ExternalInput", name=name,
                     uniquify=False)


def _mm_chain(nc, psum, lhsT_tiles, rhs_tiles):
    n = len(lhsT_tiles)
    for i in range(n):
        nc.tensor.matmul(psum, lhsT=lhsT_tiles[i], rhs=rhs_tiles[i],
                         start=(i == 0), stop=(i == n - 1))


def _build():
    nc = bacc.Bacc("TRN2", target_bir_lowering=False)
    with tile.TileContext(nc) as tc:
        _emit(nc, tc)
    nc.compile()
    return nc


def _emit(nc, tc):
    from contextlib import ExitStack

    with ExitStack() as root:
        dram = root.enter_context(tc.tile_pool(name="io", bufs=1, space="DRAM"))

        # ---------------- DRAM I/O ----------------
        x_full = _dram_in(dram, "x_full", (FULL, D), F32)
        inpT_d = _dram_in(dram, "inpT", (D, CUR), F32)
        posT_d = _dram_in(dram, "posT", (D, FULL), BF16)
        u_d = _dram_in(dram, "u_t", (P, DC), F32)
        v_d = _dram_in(dram, "v_t", (P, DC), F32)
        ln1g_d = _dram_in(dram, "ln1_g", (D,), F32)
        ln1b_d = _dram_in(dram, "ln1_b", (D,), F32)
        ln2g_d = _dram_in(dram, "ln2_g_t", (P, DC), F32)
        ln2b_d = _dram_in(dram, "ln2_b_t", (P, DC), F32)
        bkvK_d = _dram_in(dram, "bkvK_t", (P, DC), F32)
        bkvV_d = _dram_in(dram, "bkvV_row", (1, D), BF16)
        bq_d = _dram_in(dram, "bq_t", (P, DC), F32)
        bpos_d = _dram_in(dram, "bpos_t", (P, DC), F32)
        bproj_d = _dram_in(dram, "bproj_t", (P, DC), F32)
        b1_d = _dram_in(dram, "b1_t", (P, HC), F32)
        b2_d = _dram_in(dram, "b2_t", (P, DC), F32)
        nbg1_d = _dram_in(dram, "nbg1_t", (P, DC), F32)
        nbg2_d = _dram_in(dram, "nbg2_t", (P, DC), F32)

        wkv_d = _dram_in(dram, "Wkv", (D, 2 * D), BF16)
        wq_d = _dram_in(dram, "Wq", (D, D), BF16)
        wpos_d = _dram_in(dram, "Wpos", (D, D), BF16)
        wproj_d = _dram_in(dram, "Wproj", (D, D), BF16)
        gw_d = {}
        for g in (1, 2):
            for m in ("Wr", "Ur", "Wz", "Uz", "Wg", "Ug"):
                gw_d[(g, m)] = _dram_in(dram, f"g{g}_{m}", (D, D), BF16)
        w1_d = _dram_in(dram, "mlp_W1", (D, HID), BF16)
        w2_d = _dram_in(dram, "mlp_W2", (HID, D), BF16)

        out_d = dram.tile([CUR, D], F32, kind="ExternalOutput", name="out",
                          uniquify=False)

        n_scr = 8
        scr = [dram.tile([P, 1536], BF16, name=f"scr{s}") for s in range(n_scr)]

        # ---------------- constants ----------------
        const = root.enter_context(tc.tile_pool(name="const", bufs=1))
        ident_f = const.tile([P, P], F32)
        make_identity(nc, ident_f)
        ident_b = const.tile([P, P], BF16)
        make_identity(nc, ident_b)
        ones_row = const.tile([1, P], BF16)
        nc.vector.memset(ones_row, 1.0)
        ones_red = const.tile([P, 1], BF16)
        nc.vector.memset(ones_red, 1.0)
        eps_t = const.tile([P, 1], F32)
        nc.vector.memset(eps_t, EPS)

        def cload(name, dref, shape, dtype=F32):
            t = const.tile(list(shape), dtype, name=name)
            nc.sync.dma_start(out=t, in_=dref[:])
            return t

        u_sb = cload("u_sb", u_d, (P, DC))
        v_sb = cload("v_sb", v_d, (P, DC))
        ln2g_sb = cload("ln2g_sb", ln2g_d, (P, DC))
        ln2b_sb = cload("ln2b_sb", ln2b_d, (P, DC))
        bkvK_sb = cload("bkvK_sb", bkvK_d, (P, DC))
        bkvV_sb = cload("bkvV_sb", bkvV_d, (1, D), BF16)
        bq_sb = cload("bq_sb", bq_d, (P, DC))
        bpos_sb = cload("bpos_sb", bpos_d, (P, DC))
        bproj_sb = cload("bproj_sb", bproj_d, (P, DC))
        b1_sb = cload("b1_sb", b1_d, (P, HC))
        b2_sb = cload("b2_sb", b2_d, (P, DC))
        nbg1_sb = cload("nbg1_sb", nbg1_d, (P, DC))
        nbg2_sb = cload("nbg2_sb", nbg2_d, (P, DC))

        padw = const.tile([P, 512], BF16)
        nc.vector.memset(padw, NEG)
        for s in range(n_scr):
            nc.sync.dma_start(out=scr[s][:, 1024:1536], in_=padw)

        # shared psum pools (4 + 2 + 2 = 8 banks)
        psum = root.enter_context(tc.tile_pool(name="psum", bufs=4, space="PSUM"))
        psum_t = root.enter_context(tc.tile_pool(name="psum_t", bufs=2, space="PSUM"))
        psum_s = root.enter_context(tc.tile_pool(name="psum_s", bufs=2, space="PSUM"))

        def PS():
            return psum.tile([P, 512], F32, name="ps", tag="ps")

        def PT(dtype):
            return psum_t.tile([P, P], dtype, name="pt", tag="pt")

        def SM():
            return psum_s.tile([1, 512], F32, name="sm", tag="sm")

        # lifetime-managed activations (two-sided stack allocator:
        # frees must be LIFO per side, so lifetimes are laid out on
        # left/right stacks to nest properly)
        def mk(name, shape, dtype, side):
            t, fr = tc.tile(list(shape), dtype, name=name, side=side)
            return t, fr

        x1T, fr_x1T = mk("x1T", (P, DC, FULL), BF16, "left")

        # ================= Phase 1: LN1 + transpose =================
        with ExitStack() as ph:
            ln1c = ph.enter_context(tc.tile_pool(name="ln1c", bufs=1, side="left"))
            ln1g_sb = ln1c.tile([P, D], F32)
            nc.sync.dma_start(out=ln1g_sb, in_=bass.AP(
                tensor=ln1g_d.tensor, offset=ln1g_d.offset, ap=[[0, P], [1, D]]))
            ln1b_sb = ln1c.tile([P, D], F32)
            nc.sync.dma_start(out=ln1b_sb, in_=bass.AP(
                tensor=ln1b_d.tensor, offset=ln1b_d.offset, ap=[[0, P], [1, D]]))

            xw = ph.enter_context(tc.tile_pool(name="xw", bufs=3, side="left"))
            st = ph.enter_context(tc.tile_pool(name="st", bufs=3, side="left"))
            x_t = x_full[:].rearrange("(tc p) d -> p tc d", p=P)
            for tcx in range(TCF):
                xt = xw.tile([P, D], F32, name="xt")
                nc.sync.dma_start(out=xt, in_=x_t[:, tcx, :])
                stats = st.tile([P, 2, 6], F32, name="stats")
                nc.vector.bn_stats(out=stats[:, 0, :], in_=xt[:, 0:512])
                nc.vector.bn_stats(out=stats[:, 1, :], in_=xt[:, 512:1024])
                mv = st.tile([P, 2], F32, name="mv")
                nc.vector.bn_aggr(out=mv, in_=stats)
                sd = st.tile([P, 1], F32, name="sd")
                nc.scalar.activation(out=sd, in_=mv[:, 1:2], func=Act.Sqrt,
                                     bias=eps_t)
                rstd = st.tile([P, 1], F32, name="rstd")
                nc.vector.reciprocal(out=rstd, in_=sd)
                xn = xw.tile([P, D], F32, name="xn")
                nc.vector.tensor_scalar(out=xn, in0=xt, scalar1=mv[:, 0:1],
                                        scalar2=rstd, op0=AluOp.subtract,
                                        op1=AluOp.mult)
                x1n = xw.tile([P, D], F32, name="x1n")
                nc.vector.scalar_tensor_tensor(out=x1n, in0=xn, scalar=1.0,
                                               in1=ln1g_sb, op0=AluOp.mult,
                                               op1=AluOp.mult)
                nc.vector.tensor_add(x1n, x1n, ln1b_sb)
                for dc in range(DC):
                    pt = PT(F32)
                    nc.tensor.transpose(pt, x1n[:, dc * P:(dc + 1) * P], ident_f)
                    nc.vector.tensor_copy(x1T[:, dc, tcx * P:(tcx + 1) * P], pt)

        # ================= Phase 2: KT, V, qT, rT =================
        kT, fr_kT = mk("kT", (P, DC, FULL), BF16, "right")
        v_nat, fr_v = mk("v_nat", (P, TCF, D), BF16, "right")
        rT, fr_rT = mk("rT", (P, DC, FULL), BF16, "right")
        quT, fr_quT = mk("quT", (P, DC, CUR), BF16, "right")
        qvT, fr_qvT = mk("qvT", (P, DC, CUR), BF16, "right")

        with ExitStack() as ph:
            wkvp = ph.enter_context(tc.tile_pool(name="wkvp", bufs=1, side="right"))
            wkv = wkvp.tile([P, DC, 2 * D], BF16)
            nc.sync.dma_start(out=wkv, in_=wkv_d[:].rearrange("(kc p) n -> p kc n", p=P))
            for n in range(DC):
                for th in range(2):
                    ps = PS()
                    _mm_chain(nc, ps,
                              [wkv[:, k, n * P:(n + 1) * P] for k in range(DC)],
                              [x1T[:, k, th * 512:(th + 1) * 512] for k in range(DC)])
                    nc.vector.tensor_scalar_add(kT[:, n, th * 512:(th + 1) * 512],
                                                ps, bkvK_sb[:, n:n + 1])
            for t in range(TCF):
                for nh in range(2):
                    ps = PS()
                    for k in range(DC):
                        nc.tensor.matmul(ps, lhsT=x1T[:, k, t * P:(t + 1) * P],
                                         rhs=wkv[:, k, D + nh * 512:D + (nh + 1) * 512],
                                         start=(k == 0), stop=False)
                    nc.tensor.matmul(ps, lhsT=ones_row,
                                     rhs=bkvV_sb[:, nh * 512:(nh + 1) * 512],
                                     start=False, stop=True)
                    nc.vector.tensor_copy(v_nat[:, t, nh * 512:(nh + 1) * 512], ps)
        with ExitStack() as ph:
            wqp = ph.enter_context(tc.tile_pool(name="wqp", bufs=1, side="right"))
            wq = wqp.tile([P, DC, D], BF16)
            nc.sync.dma_start(out=wq, in_=wq_d[:].rearrange("(kc p) n -> p kc n", p=P))
            qw = ph.enter_context(tc.tile_pool(name="qw", bufs=3, side="right"))
            for n in range(DC):
                ps = PS()
                _mm_chain(nc, ps,
                          [wq[:, k, n * P:(n + 1) * P] for k in range(DC)],
                          [x1T[:, k, CUR:FULL] for k in range(DC)])
                qn = qw.tile([P, 512], F32, name="qn")
                nc.vector.tensor_scalar_add(qn, ps, bq_sb[:, n:n + 1])
                nc.vector.tensor_scalar_add(quT[:, n, :], qn, u_sb[:, n:n + 1])
                nc.vector.tensor_scalar_add(qvT[:, n, :], qn, v_sb[:, n:n + 1])
        with ExitStack() as ph:
            wpp = ph.enter_context(tc.tile_pool(name="wpp", bufs=1, side="right"))
            wpos = wpp.tile([P, DC, D], BF16)
            nc.sync.dma_start(out=wpos, in_=wpos_d[:].rearrange("(kc p) n -> p kc n", p=P))
            posT_sb = wpp.tile([P, DC, FULL], BF16)
            nc.sync.dma_start(out=posT_sb, in_=posT_d[:].rearrange("(kc p) f -> p kc f", p=P))
            for n in range(DC):
                for fh in range(2):
                    ps = PS()
                    _mm_chain(nc, ps,
                              [wpos[:, k, n * P:(n + 1) * P] for k in range(DC)],
                              [posT_sb[:, k, fh * 512:(fh + 1) * 512] for k in range(DC)])
                    nc.vector.tensor_scalar_add(rT[:, n, fh * 512:(fh + 1) * 512],
                                                ps, bpos_sb[:, n:n + 1])
        fr_x1T()

        # ================= Phase 3: attention =================
        avT, fr_avT = mk("avT", (P, DC, CUR), BF16, "left")
        with ExitStack() as ph:
            aw = ph.enter_context(tc.tile_pool(name="aw", bufs=3, side="left"))
            atw = ph.enter_context(tc.tile_pool(name="atw", bufs=2, side="left"))
            rw = ph.enter_context(tc.tile_pool(name="rw", bufs=2, side="left"))
            scnt = 0
            for h in range(HEAD_NUM):
                ch, rb = h // 2, (h % 2) * HEAD_DIM
                quh = quT[rb:rb + HEAD_DIM, ch, :]
                qvh = qvT[rb:rb + HEAD_DIM, ch, :]
                kh = kT[rb:rb + HEAD_DIM, ch, :]
                rh = rT[rb:rb + HEAD_DIM, ch, :]
                attnT = atw.tile([P, TCF, 512], BF16, name="attnT")
                shps = []
                for ic in range(TCC):
                    s_t = scr[scnt % n_scr]
                    scnt += 1
                    pb = aw.tile([P, FULL], BF16, name="pb", bufs=4)
                    for jh in range(2):
                        pp = PS()
                        nc.tensor.matmul(pp, lhsT=qvh[:, ic * P:(ic + 1) * P],
                                         rhs=rh[:, jh * 512:(jh + 1) * 512],
                                         start=True, stop=True)
                        nc.scalar.copy(pb[:, jh * 512:(jh + 1) * 512], pp)
                    nc.sync.dma_start(out=s_t[:, 0:1024], in_=pb)
                    shp = aw.tile([P, FULL], BF16, name="shp", bufs=5)
                    shift_ap = bass.AP(tensor=s_t.tensor, offset=s_t.offset + 511,
                                       ap=[[1535, P], [1, FULL]])
                    nc.sync.dma_start(out=shp, in_=shift_ap)
                    shps.append(shp)
                for ic in range(TCC):
                    shp = shps[ic]
                    es = aw.tile([P, FULL], BF16, name="es")
                    for jh in range(2):
                        cp = PS()
                        nc.tensor.matmul(cp, lhsT=quh[:, ic * P:(ic + 1) * P],
                                         rhs=kh[:, jh * 512:(jh + 1) * 512],
                                         start=True, stop=True)
                        sm = aw.tile([P, 512], F32, name="smadd")
                        nc.vector.tensor_add(sm, cp, shp[:, jh * 512:(jh + 1) * 512])
                        nc.scalar.activation(out=es[:, jh * 512:(jh + 1) * 512],
                                             in_=sm, func=Act.Exp, scale=SCALE)
                    for jc in range(TCF):
                        if jc > ic + 4:
                            nc.vector.memset(attnT[:, jc, ic * P:(ic + 1) * P], 0.0)
                            continue
                        pt = PT(BF16)
                        nc.tensor.transpose(pt, es[:, jc * P:(jc + 1) * P], ident_b)
                        if jc % 2 == 0:
                            nc.vector.tensor_copy(attnT[:, jc, ic * P:(ic + 1) * P], pt)
                        else:
                            nc.scalar.copy(attnT[:, jc, ic * P:(ic + 1) * P], pt)
                dn = SM()
                _mm_chain(nc, dn, [ones_red] * TCF,
                          [attnT[:, jc, :] for jc in range(TCF)])
                recip = rw.tile([1, 512], F32, name="recip")
                nc.vector.reciprocal(out=recip, in_=dn)
                recipB = rw.tile([HEAD_DIM, 512], F32, name="recipB")
                nc.gpsimd.partition_broadcast(recipB, recip)
                av = PS()
                for jc in range(TCF):
                    nc.tensor.matmul(av[0:HEAD_DIM, :],
                                     lhsT=v_nat[:, jc, h * HEAD_DIM:(h + 1) * HEAD_DIM],
                                     rhs=attnT[:, jc, :],
                                     start=(jc == 0), stop=(jc == TCF - 1))
                nc.vector.tensor_mul(avT[rb:rb + HEAD_DIM, ch, :],
                                     av[0:HEAD_DIM, :], recipB)
        fr_qvT(); fr_quT(); fr_rT(); fr_v(); fr_kT()

        # ================= Phase 4: proj + GRU1 =================
        a1T, fr_a1T = mk("a1T", (P, DC, CUR), BF16, "right")
        with ExitStack() as ph:
            wpr = ph.enter_context(tc.tile_pool(name="wpr", bufs=1, side="left"))
            wproj = wpr.tile([P, DC, D], BF16)
            nc.sync.dma_start(out=wproj, in_=wproj_d[:].rearrange("(kc p) n -> p kc n", p=P))
            for n in range(DC):
                ps = PS()
                _mm_chain(nc, ps,
                          [wproj[:, k, n * P:(n + 1) * P] for k in range(DC)],
                          [avT[:, k, :] for k in range(DC)])
                nc.vector.tensor_scalar(out=a1T[:, n, :], in0=ps,
                                        scalar1=bproj_sb[:, n:n + 1],
                                        scalar2=0.0, op0=AluOp.add,
                                        op1=AluOp.max)
        fr_avT()

        o1T_f, fr_o1f = mk("o1T_f", (P, DC, CUR), F32, "left")
        o1T_b, fr_o1b = mk("o1T_b", (P, DC, CUR), BF16, "left")
        inpT_f, fr_inpf = mk("inpT_f", (P, DC, CUR), F32, "left")
        inpT_b, fr_inpb = mk("inpT_b", (P, DC, CUR), BF16, "left")
        nc.sync.dma_start(out=inpT_f, in_=inpT_d[:].rearrange("(kc p) t -> p kc t", p=P))
        nc.vector.tensor_copy(inpT_b, inpT_f)
        with ExitStack() as ph:
            _gru(nc, tc, ph, PS, gw_d, 1, a1T, inpT_b, inpT_f, nbg1_sb,
                 o1T_f, o1T_b)
        fr_inpb(); fr_inpf(); fr_a1T()

        # ================= Phase 5: LN2 =================
        x2T, fr_x2T = mk("x2T", (P, DC, CUR), BF16, "right")
        with ExitStack() as ph:
            lw = ph.enter_context(tc.tile_pool(name="lw", bufs=2, side="left"))
            sqp = ph.enter_context(tc.tile_pool(name="sqp", bufs=1, side="left"))
            sq = sqp.tile([P, DC, 512], BF16, name="sq")
            for n in range(DC):
                nc.vector.tensor_mul(sq[:, n, :], o1T_f[:, n, :], o1T_f[:, n, :])
            s1 = SM()
            _mm_chain(nc, s1, [ones_red] * DC, [o1T_b[:, n, :] for n in range(DC)])
            mean = lw.tile([1, 512], F32, name="mean")
            nc.vector.tensor_scalar_mul(mean, s1, 1.0 / D)
            s2 = SM()
            _mm_chain(nc, s2, [ones_red] * DC, [sq[:, n, :] for n in range(DC)])
            m2m = lw.tile([1, 512], F32, name="m2m")
            nc.vector.tensor_scalar_mul(m2m, s2, 1.0 / D)
            var = lw.tile([1, 512], F32, name="var")
            nc.vector.scalar_tensor_tensor(out=var, in0=mean, scalar=1.0,
                                           in1=mean, op0=AluOp.mult,
                                           op1=AluOp.mult)
            nc.vector.tensor_sub(var, m2m, var)
            sd = lw.tile([1, 512], F32, name="sd2")
            nc.scalar.activation(out=sd, in_=var, func=Act.Sqrt,
                                 bias=eps_t[0:1, :])
            rstd = lw.tile([1, 512], F32, name="rstd2")
            nc.vector.reciprocal(out=rstd, in_=sd)
            meanB = lw.tile([P, 512], F32, name="meanB")
            nc.gpsimd.partition_broadcast(meanB, mean)
            rstdB = lw.tile([P, 512], F32, name="rstdB")
            nc.gpsimd.partition_broadcast(rstdB, rstd)
            for n in range(DC):
                t1 = lw.tile([P, 512], F32, name="t1")
                nc.vector.tensor_sub(t1, o1T_f[:, n, :], meanB)
                nc.vector.tensor_mul(t1, t1, rstdB)
                nc.vector.tensor_scalar(out=x2T[:, n, :], in0=t1,
                                        scalar1=ln2g_sb[:, n:n + 1],
                                        scalar2=ln2b_sb[:, n:n + 1],
                                        op0=AluOp.mult, op1=AluOp.add)

        # ================= Phase 6: MLP =================
        with ExitStack() as ph6:
            m1w = ph6.enter_context(tc.tile_pool(name="m1w", bufs=1, side="right"))
            m1T = m1w.tile([P, HC, 512], BF16)
            with ExitStack() as ph:
                w1p = ph.enter_context(tc.tile_pool(name="w1p", bufs=4, side="right"))
                w1r = w1_d[:].rearrange("(kc p) n -> p kc n", p=P)
                for n in range(HC):
                    w1t = w1p.tile([P, DC, P], BF16, name="w1t", tag="w1t")
                    nc.sync.dma_start(out=w1t, in_=w1r[:, :, n * P:(n + 1) * P])
                    ps = PS()
                    _mm_chain(nc, ps,
                              [w1t[:, k, :] for k in range(DC)],
                              [x2T[:, k, :] for k in range(DC)])
                    nc.vector.tensor_scalar(out=m1T[:, n, :], in0=ps,
                                            scalar1=b1_sb[:, n:n + 1],
                                            scalar2=0.0, op0=AluOp.add,
                                            op1=AluOp.max)
            m2T, fr_m2T = mk("m2T", (P, DC, CUR), BF16, "left")
            w2p = ph6.enter_context(tc.tile_pool(name="w2p", bufs=3, side="left"))
            w2r = w2_d[:].rearrange("(kc p) n -> p kc n", p=P)
            for n in range(DC):
                w2t = w2p.tile([P, HC, P], BF16, name="w2t", tag="w2t")
                nc.sync.dma_start(out=w2t, in_=w2r[:, :, n * P:(n + 1) * P])
                ps = PS()
                _mm_chain(nc, ps,
                          [w2t[:, k, :] for k in range(HC)],
                          [m1T[:, k, :] for k in range(HC)])
                nc.vector.tensor_scalar(out=m2T[:, n, :], in0=ps,
                                        scalar1=b2_sb[:, n:n + 1],
                                        scalar2=0.0, op0=AluOp.add,
                                        op1=AluOp.max)
        fr_x2T()

        # ================= Phase 7: GRU2 =================
        o2T_f, fr_o2 = mk("o2T_f", (P, DC, CUR), F32, "right")
        with ExitStack() as ph:
            _gru(nc, tc, ph, PS, gw_d, 2, m2T, o1T_b, o1T_f, nbg2_sb,
                 o2T_f, None)
        fr_m2T(); fr_o1b(); fr_o1f()

        # ================= Phase 8: transpose out =================
        with ExitStack() as ph:
            ow = ph.enter_context(tc.tile_pool(name="ow", bufs=2, side="left"))
            for t in range(TCC):
                on = ow.tile([P, D], F32, name="on")
                for n in range(DC):
                    pt = PT(F32)
                    nc.tensor.transpose(pt, o2T_f[:, n, t * P:(t + 1) * P], ident_f)
                    nc.vector.tensor_copy(on[:, n * P:(n + 1) * P], pt)
                nc.sync.dma_start(out=out_d[t * P:(t + 1) * P, :], in_=on)
        fr_o2()


def _gru(nc, tc, ph, PS, gw_d, g, yT, xT_b, xT_f, nbg_sb, oT_f, oT_b):
    gwp = ph.enter_context(tc.tile_pool(name=f"gw{g}", bufs=3, side="left"))
    gtmp = ph.enter_context(tc.tile_pool(name=f"gt{g}", bufs=2, side="left"))
    gper = ph.enter_context(tc.tile_pool(name=f"gp{g}", bufs=1, side="left"))

    def loadw(m):
        w = gwp.tile([P, DC, D], BF16, name=f"gwt_{m}", tag="gwt")
        nc.sync.dma_start(out=w, in_=gw_d[(g, m)][:].rearrange("(kc p) n -> p kc n", p=P))
        return w

    wr, ur = loadw("Wr"), loadw("Ur")
    rx = gper.tile([P, DC, 512], BF16, name="rx")
    for n in range(DC):
        ps = PS()
        for k in range(DC):
            nc.tensor.matmul(ps, lhsT=wr[:, k, n * P:(n + 1) * P],
                             rhs=yT[:, k, :], start=(k == 0), stop=False)
        for k in range(DC):
            nc.tensor.matmul(ps, lhsT=ur[:, k, n * P:(n + 1) * P],
                             rhs=xT_b[:, k, :], start=False, stop=(k == DC - 1))
        rr = gtmp.tile([P, 512], F32, name="rr")
        nc.scalar.activation(out=rr, in_=ps, func=Act.Sigmoid)
        nc.vector.tensor_mul(rx[:, n, :], rr, xT_f[:, n, :])
    wz, uz = loadw("Wz"), loadw("Uz")
    zt = gper.tile([P, DC, 512], F32, name="zt")
    for n in range(DC):
        ps = PS()
        for k in range(DC):
            nc.tensor.matmul(ps, lhsT=wz[:, k, n * P:(n + 1) * P],
                             rhs=yT[:, k, :], start=(k == 0), stop=False)
        for k in range(DC):
            nc.tensor.matmul(ps, lhsT=uz[:, k, n * P:(n + 1) * P],
                             rhs=xT_b[:, k, :], start=False, stop=(k == DC - 1))
        nc.scalar.activation(out=zt[:, n, :], in_=ps, func=Act.Sigmoid,
                             bias=nbg_sb[:, n:n + 1])
    wg, ug = loadw("Wg"), loadw("Ug")
    for n in range(DC):
        ps = PS()
        for k in range(DC):
            nc.tensor.matmul(ps, lhsT=wg[:, k, n * P:(n + 1) * P],
                             rhs=yT[:, k, :], start=(k == 0), stop=False)
        for k in range(DC):
            nc.tensor.matmul(ps, lhsT=ug[:, k, n * P:(n + 1) * P],
                             rhs=rx[:, k, :], start=False, stop=(k == DC - 1))
        ht = gtmp.tile([P, 512], F32, name="ht")
        nc.scalar.activation(out=ht, in_=ps, func=Act.Tanh)
        nc.vector.tensor_sub(ht, ht, xT_f[:, n, :])
        nc.vector.tensor_mul(ht, ht, zt[:, n, :])
        nc.vector.tensor_add(oT_f[:, n, :], ht, xT_f[:, n, :])
        if oT_b is not None:
            nc.vector.tensor_copy(oT_b[:, n, :], oT_f[:, n, :])


_NC_CACHE = {}


def _get_nc():
    if "nc" not in _NC_CACHE:
        _NC_CACHE["nc"] = _build()
    return _NC_CACHE["nc"]


def _chunk_t(vec):
    n = vec.shape[0] // P
    return np.ascontiguousarray(vec.reshape(n, P).T.astype(np.float32))


def _prep(inputs):
    f32 = np.float32
    bf = ml_dtypes.bfloat16
    inp = np.asarray(inputs["inputs"], f32)
    mem = np.asarray(inputs["memory"], f32)
    pos = np.asarray(inputs["pos_embedding"], f32)[:, 0, :]

    shared = {
        "posT": np.ascontiguousarray(pos.T).astype(bf),
        "u_t": _chunk_t(np.asarray(inputs["u"], f32).reshape(-1)),
        "v_t": _chunk_t(np.asarray(inputs["v"], f32).reshape(-1)),
        "ln1_g": np.asarray(inputs["ln1_g"], f32),
        "ln1_b": np.asarray(inputs["ln1_b"], f32),
        "ln2_g_t": _chunk_t(np.asarray(inputs["ln2_g"], f32)),
        "ln2_b_t": _chunk_t(np.asarray(inputs["ln2_b"], f32)),
        "bkvK_t": _chunk_t(np.asarray(inputs["bkv"], f32)[0:D]),
        "bkvV_row": np.asarray(inputs["bkv"], f32)[D:2 * D].reshape(1, D).astype(bf),
        "bq_t": _chunk_t(np.asarray(inputs["bq"], f32)),
        "bpos_t": _chunk_t(np.asarray(inputs["bpos"], f32)),
        "bproj_t": _chunk_t(np.asarray(inputs["bproj"], f32)),
        "b1_t": _chunk_t(np.asarray(inputs["mlp_b1"], f32)),
        "b2_t": _chunk_t(np.asarray(inputs["mlp_b2"], f32)),
        "nbg1_t": _chunk_t(-np.asarray(inputs["g1_bg"], f32)),
        "nbg2_t": _chunk_t(-np.asarray(inputs["g2_bg"], f32)),
        "Wkv": np.asarray(inputs["Wkv"], f32).astype(bf),
        "Wq": np.asarray(inputs["Wq"], f32).astype(bf),
        "Wpos": np.asarray(inputs["Wpos"], f32).astype(bf),
        "Wproj": np.asarray(inputs["Wproj"], f32).astype(bf),
        "mlp_W1": np.asarray(inputs["mlp_W1"], f32).astype(bf),
        "mlp_W2": np.asarray(inputs["mlp_W2"], f32).astype(bf),
    }
    for g in (1, 2):
        for m in ("Wr", "Ur", "Wz", "Uz", "Wg", "Ug"):
            shared[f"g{g}_{m}"] = np.asarray(inputs[f"g{g}_{m}"], f32).astype(bf)

    in_maps = []
    for b in range(BS):
        im = dict(shared)
        im["x_full"] = np.ascontiguousarray(
            np.concatenate([mem[:, b, :], inp[:, b, :]], axis=0))
        im["inpT"] = np.ascontiguousarray(inp[:, b, :].T)
        in_maps.append(im)
    return in_maps


def kernel(**inputs):
    nc = _get_nc()
    in_maps = _prep(inputs)
    res = run_bass_kernel_spmd(nc, in_maps, core_ids=list(range(BS)))
    out = np.stack([res.results[b]["out"] for b in range(BS)], axis=1)
    return np.ascontiguousarray(out.astype(np.float32))


if __name__ == "__main__":
    _get_nc()
    print("build+compile OK")



# revision 11
# speedup vs baseline: 1.7610x; 1.7610x over previous
"""GTrXL layer (TransformerXL attention + GRU gating) on 8 TRN2 NeuronCores.

Sharding: pure data-parallel over batch (BS=8 -> 1 batch element per core).
No collectives. Per-core Bass/Tile kernel computes the full layer for its
batch element.

Layout convention on-chip: activations are kept TRANSPOSED [feature, token]
(feature on partitions, 128-chunks).

Precision strategy: all big matmuls run in fp8-e4m3 with DoubleRow perf mode
(2 contraction k-tiles per pass -> 2x bf16 throughput). Weights are scaled by
256 on the host before fp8 quantization (keeps values out of the subnormal
range); every PSUM consumer applies a 2^-8 scale. Activations quantize to fp8
at natural scale; softmax weights are scaled by 128 (fp8 S=7) before the AV
matmul and the output rescaled by 2^-7. Elementwise math (LN, GRU combine)
stays f32; logits/es stay bf16.

Relative-shift: pos scores P[i, rel] are computed only for the needed rel
range [384-128*ic, 1024) per 128-query chunk, written to a DRAM scratch of
row stride 1536 whose tail 512 columns are pre-filled with -1e30. The shifted
read  shifted[i, j] = P[i, 511 - 128*ic + j - i]  is a single strided DMA
(row step 1535, per-chunk offset 511-128*ic — this is the CORRECT global
TrXL shift), and the pad lands exactly on the masked region j > i + 512 + 128*ic.

Softmax denominators come free from the exp instruction's accum_out; the
reciprocal is folded into the es -> fp8 normalization (pre-transpose).
"""

import sys

if '/opt/trn_rl_repo' not in sys.path:
    sys.path.insert(0, '/opt/trn_rl_repo')

import numpy as np
import ml_dtypes

import concourse.bass as bass
import concourse.tile as tile
from concourse import bacc, mybir
from concourse.bass_utils import run_bass_kernel_spmd
from concourse.masks import make_identity

BF16 = mybir.dt.bfloat16
F32 = mybir.dt.float32
FP8 = mybir.dt.float8e4

HEAD_NUM, HEAD_DIM = 16, 64
D, HID = 1024, 4096
CUR, PREV, BS = 512, 512, 8
FULL = CUR + PREV
EPS = 1e-5
SCALE = 1.0 / (HEAD_DIM ** 0.5)
P = 128
DC = D // P          # 8 feature chunks
HC = HID // P        # 32 hidden chunks
TCF = FULL // P      # 8 full-token chunks
TCC = CUR // P       # 4 query-token chunks
NEG = -1.0e30
WS = 256.0           # host-side weight scale before fp8 quantization
ISW = 1.0 / WS       # psum de-scale
ES_S = 128.0         # softmax-weight fp8 scale
IES = 1.0 / ES_S

AluOp = mybir.AluOpType
Act = mybir.ActivationFunctionType
DR = mybir.MatmulPerfMode.DoubleRow


def _dram_in(dram, name, shape, dtype):
    return dram.tile(list(shape), dtype, kind="ExternalInput", name=name,
                     uniquify=False)


def _dr_quads(nc, ps, segs):
    """Fill psum [128, 512] via DoubleRow quadrant chains.

    segs: list of (w, x, c0, t0, kpairs) — accumulate over all segs:
      ps[n, t] += sum_k w[k, c0+n] * x[k, t0+t]   (k over kpairs*256 lanes)
    w, x are [P, 2*kpairs.., *] fp8 tiles (chunk dim second).
    """
    for nh in range(2):
        for qh in range(2):
            out = ps[nh * 64:nh * 64 + 64, qh * 256:qh * 256 + 256]
            total = sum(len(s[4]) for s in segs)
            i = 0
            for (w, x, c0, t0, kpairs) in segs:
                for m in kpairs:
                    nc.tensor.matmul(
                        out,
                        lhsT=w[:, 2 * m:2 * m + 2,
                               c0 + nh * 64:c0 + nh * 64 + 64],
                        rhs=x[:, 2 * m:2 * m + 2,
                              t0 + qh * 256:t0 + qh * 256 + 256],
                        perf_mode=DR,
                        start=(i == 0), stop=(i == total - 1))
                    i += 1


def _build():
    nc = bacc.Bacc("TRN2", target_bir_lowering=False)
    with tile.TileContext(nc) as tc:
        _emit(nc, tc)
    nc.compile()
    return nc


def _emit(nc, tc):
    from contextlib import ExitStack

    with ExitStack() as root:
        dram = root.enter_context(tc.tile_pool(name="io", bufs=1, space="DRAM"))

        # ---------------- DRAM I/O ----------------
        x_full = _dram_in(dram, "x_full", (FULL, D), F32)
        inpT_d = _dram_in(dram, "inpT", (D, CUR), F32)
        posT_d = _dram_in(dram, "posT8", (D, FULL), FP8)
        u_d = _dram_in(dram, "u_t", (P, DC), F32)
        v_d = _dram_in(dram, "v_t", (P, DC), F32)
        ln1g_d = _dram_in(dram, "ln1_g", (D,), F32)
        ln1b_d = _dram_in(dram, "ln1_b", (D,), F32)
        ln2g_d = _dram_in(dram, "ln2_g_t", (P, DC), F32)
        ln2b_d = _dram_in(dram, "ln2_b_t", (P, DC), F32)
        bkvK_d = _dram_in(dram, "bkvK_t", (P, DC), F32)
        bkvV_d = _dram_in(dram, "bkvV_row", (1, D), F32)
        bq_d = _dram_in(dram, "bq_t", (P, DC), F32)
        bpos_d = _dram_in(dram, "bpos_t", (P, DC), F32)
        bproj_d = _dram_in(dram, "bproj_t", (P, DC), F32)
        b1_d = _dram_in(dram, "b1_t", (P, HC), F32)
        b2_d = _dram_in(dram, "b2_t", (P, DC), F32)
        nbg1_d = _dram_in(dram, "nbg1_t", (P, DC), F32)
        nbg2_d = _dram_in(dram, "nbg2_t", (P, DC), F32)

        wkvK_d = _dram_in(dram, "WkvK8", (D, D), FP8)
        wkvV_d = _dram_in(dram, "WkvV8", (D, D), FP8)
        wq_d = _dram_in(dram, "Wq8", (D, D), FP8)
        wpos_d = _dram_in(dram, "Wpos8", (D, D), FP8)
        wproj_d = _dram_in(dram, "Wproj8", (D, D), FP8)
        gw_d = {}
        for g in (1, 2):
            for m in ("Wr", "Ur", "Wz", "Uz", "Wg", "Ug"):
                gw_d[(g, m)] = _dram_in(dram, f"g{g}_{m}8", (D, D), FP8)
        w1_d = _dram_in(dram, "mlp_W18", (D, HID), FP8)
        w2_d = _dram_in(dram, "mlp_W28", (HID, D), FP8)

        # transposed output [D, CUR]; host transposes back
        out_d = dram.tile([D, CUR], F32, kind="ExternalOutput", name="out",
                          uniquify=False)

        n_scr = 8
        scr = [dram.tile([P, 1536], BF16, name=f"scr{s}") for s in range(n_scr)]

        # ---------------- constants ----------------
        const = root.enter_context(tc.tile_pool(name="const", bufs=1))
        ident_f = const.tile([P, P], F32)
        make_identity(nc, ident_f)
        ident_8 = const.tile([P, P], FP8)
        make_identity(nc, ident_8)
        ones_red8 = const.tile([P, 1], FP8)
        nc.vector.memset(ones_red8, 1.0)
        eps_t = const.tile([P, 1], F32)
        nc.vector.memset(eps_t, EPS)

        def cload(name, dref, shape, dtype=F32):
            t = const.tile(list(shape), dtype, name=name)
            nc.sync.dma_start(out=t, in_=dref[:])
            return t

        u_sb = cload("u_sb", u_d, (P, DC))
        v_sb = cload("v_sb", v_d, (P, DC))
        ln2g_sb = cload("ln2g_sb", ln2g_d, (P, DC))
        ln2b_sb = cload("ln2b_sb", ln2b_d, (P, DC))
        bkvK_sb = cload("bkvK_sb", bkvK_d, (P, DC))
        bq_sb = cload("bq_sb", bq_d, (P, DC))
        bpos_sb = cload("bpos_sb", bpos_d, (P, DC))
        bproj_sb = cload("bproj_sb", bproj_d, (P, DC))
        b1_sb = cload("b1_sb", b1_d, (P, HC))
        b2_sb = cload("b2_sb", b2_d, (P, DC))
        nbg1_sb = cload("nbg1_sb", nbg1_d, (P, DC))
        nbg2_sb = cload("nbg2_sb", nbg2_d, (P, DC))
        # V bias broadcast to all partitions (free-dim varying)
        bvV_sb = const.tile([P, D], F32, name="bvV_sb")
        nc.sync.dma_start(out=bvV_sb, in_=bass.AP(
            tensor=bkvV_d.tensor, offset=bkvV_d.offset, ap=[[0, P], [1, D]]))

        padw = const.tile([P, 512], BF16)
        nc.vector.memset(padw, NEG)
        for s in range(n_scr):
            nc.sync.dma_start(out=scr[s][:, 1024:1536], in_=padw)

        # engine rotation for copies / elementwise
        vecs = [nc.vector, nc.gpsimd]

        def VE(i):
            return vecs[i % 2]

        # phase-scoped psum pools (PSUM is only 8 banks; attention needs them)
        psum_box = {}

        def PS():
            return psum_box["p"].tile([P, 512], F32, name="ps", tag="ps")

        def SM():
            return psum_box["s"].tile([1, 512], F32, name="sm", tag="sm")

        def mk(name, shape, dtype, side):
            t, fr = tc.tile(list(shape), dtype, name=name, side=side)
            return t, fr

        x1T8, fr_x1T = mk("x1T8", (P, DC, FULL), FP8, "left")

        # ================= Phase 1: LN1 + transpose =================
        with ExitStack() as ph:
            ln1c = ph.enter_context(tc.tile_pool(name="ln1c", bufs=1, side="left"))
            ln1g_sb = ln1c.tile([P, D], F32)
            nc.sync.dma_start(out=ln1g_sb, in_=bass.AP(
                tensor=ln1g_d.tensor, offset=ln1g_d.offset, ap=[[0, P], [1, D]]))
            ln1b_sb = ln1c.tile([P, D], F32)
            nc.sync.dma_start(out=ln1b_sb, in_=bass.AP(
                tensor=ln1b_d.tensor, offset=ln1b_d.offset, ap=[[0, P], [1, D]]))

            xw = ph.enter_context(tc.tile_pool(name="xw", bufs=3, side="left"))
            st = ph.enter_context(tc.tile_pool(name="st", bufs=3, side="left"))
            ptp = ph.enter_context(tc.tile_pool(name="ptp", bufs=2, space="PSUM"))
            x_t = x_full[:].rearrange("(tc p) d -> p tc d", p=P)
            for tcx in range(TCF):
                xt = xw.tile([P, D], F32, name="xt")
                nc.sync.dma_start(out=xt, in_=x_t[:, tcx, :])
                stats = st.tile([P, 2, 6], F32, name="stats")
                nc.vector.bn_stats(out=stats[:, 0, :], in_=xt[:, 0:512])
                nc.vector.bn_stats(out=stats[:, 1, :], in_=xt[:, 512:1024])
                mv = st.tile([P, 2], F32, name="mv")
                nc.vector.bn_aggr(out=mv, in_=stats)
                sd = st.tile([P, 1], F32, name="sd")
                nc.scalar.activation(out=sd, in_=mv[:, 1:2], func=Act.Sqrt,
                                     bias=eps_t)
                rstd = st.tile([P, 1], F32, name="rstd")
                nc.vector.reciprocal(out=rstd, in_=sd)
                xn = xw.tile([P, D], F32, name="xn")
                nc.vector.tensor_scalar(out=xn, in0=xt, scalar1=mv[:, 0:1],
                                        scalar2=rstd, op0=AluOp.subtract,
                                        op1=AluOp.mult)
                x1n = xw.tile([P, D], F32, name="x1n")
                nc.vector.scalar_tensor_tensor(out=x1n, in0=xn, scalar=1.0,
                                               in1=ln1g_sb, op0=AluOp.mult,
                                               op1=AluOp.mult)
                nc.vector.tensor_add(x1n, x1n, ln1b_sb)
                for dc in range(DC):
                    pt = ptp.tile([P, P], F32, name="pt1", tag="pt1")
                    nc.tensor.transpose(pt, x1n[:, dc * P:(dc + 1) * P], ident_f)
                    VE(dc).tensor_copy(x1T8[:, dc, tcx * P:(tcx + 1) * P], pt)

        # ================= Phase 2: kT, V, qT, rT (all fp8 DoubleRow) ========
        kT, fr_kT = mk("kT", (P, DC, FULL), FP8, "right")
        v_nat, fr_v = mk("v_nat", (P, TCF, D), FP8, "right")
        rT, fr_rT = mk("rT", (P, DC, FULL), FP8, "right")
        quT, fr_quT = mk("quT", (P, DC, CUR), FP8, "right")
        qvT, fr_qvT = mk("qvT", (P, DC, CUR), FP8, "right")

        KP = [0, 1, 2, 3]  # the 4 k-chunk pairs covering D=1024

        with ExitStack() as ph:
            psum_box["p"] = ph.enter_context(
                tc.tile_pool(name="psum2a", bufs=4, space="PSUM"))
            wkvp = ph.enter_context(tc.tile_pool(name="wkvp", bufs=1, side="right"))
            wkvK = wkvp.tile([P, DC, D], FP8)
            nc.scalar.dma_start(out=wkvK, in_=wkvK_d[:].rearrange("(kc p) n -> p kc n", p=P))
            wkvV = wkvp.tile([P, DC, D], FP8)
            nc.scalar.dma_start(out=wkvV, in_=wkvV_d[:].rearrange("(kc p) n -> p kc n", p=P))
            for n in range(DC):
                for th in range(2):
                    ps = PS()
                    _dr_quads(nc, ps, [(wkvK, x1T8, n * P, th * 512, KP)])
                    VE(n + th).tensor_scalar(
                        out=kT[:, n, th * 512:(th + 1) * 512], in0=ps,
                        scalar1=ISW, scalar2=bkvK_sb[:, n:n + 1],
                        op0=AluOp.mult, op1=AluOp.add)
            for t in range(TCF):
                for nh in range(2):
                    ps = PS()
                    _dr_quads(nc, ps, [(x1T8, wkvV, t * P, nh * 512, KP)])
                    VE(t + nh).scalar_tensor_tensor(
                        out=v_nat[:, t, nh * 512:(nh + 1) * 512], in0=ps,
                        scalar=ISW, in1=bvV_sb[:, nh * 512:(nh + 1) * 512],
                        op0=AluOp.mult, op1=AluOp.add)
        with ExitStack() as ph:
            psum_box["p"] = ph.enter_context(
                tc.tile_pool(name="psum2b", bufs=4, space="PSUM"))
            wqp = ph.enter_context(tc.tile_pool(name="wqp", bufs=1, side="right"))
            wq = wqp.tile([P, DC, D], FP8)
            nc.scalar.dma_start(out=wq, in_=wq_d[:].rearrange("(kc p) n -> p kc n", p=P))
            wpos = wqp.tile([P, DC, D], FP8)
            nc.scalar.dma_start(out=wpos, in_=wpos_d[:].rearrange("(kc p) n -> p kc n", p=P))
            posT_sb = wqp.tile([P, DC, FULL], FP8)
            nc.scalar.dma_start(out=posT_sb, in_=posT_d[:].rearrange("(kc p) f -> p kc f", p=P))
            qw = ph.enter_context(tc.tile_pool(name="qw", bufs=3, side="right"))
            for n in range(DC):
                ps = PS()
                _dr_quads(nc, ps, [(wq, x1T8, n * P, CUR, KP)])
                qn = qw.tile([P, 512], F32, name="qn")
                nc.vector.tensor_scalar(out=qn, in0=ps, scalar1=ISW,
                                        scalar2=bq_sb[:, n:n + 1],
                                        op0=AluOp.mult, op1=AluOp.add)
                nc.vector.tensor_scalar_add(quT[:, n, :], qn, u_sb[:, n:n + 1])
                nc.gpsimd.tensor_scalar_add(qvT[:, n, :], qn, v_sb[:, n:n + 1])
            for n in range(DC):
                for fh in range(2):
                    ps = PS()
                    _dr_quads(nc, ps, [(wpos, posT_sb, n * P, fh * 512, KP)])
                    VE(n + fh).tensor_scalar(
                        out=rT[:, n, fh * 512:(fh + 1) * 512], in0=ps,
                        scalar1=ISW, scalar2=bpos_sb[:, n:n + 1],
                        op0=AluOp.mult, op1=AluOp.add)
        fr_x1T()

        # prefetch proj + GRU1 first weights on the ACT hwdge queue
        wprp = root.enter_context(tc.tile_pool(name="wprp", bufs=1, side="left"))
        wproj = wprp.tile([P, DC, D], FP8)
        nc.scalar.dma_start(out=wproj, in_=wproj_d[:].rearrange("(kc p) n -> p kc n", p=P))

        # reserve GRU output tiles below the inp tiles (LIFO frees)
        o1T_f, fr_o1f = mk("o1T_f", (P, DC, CUR), F32, "left")
        o1_8, fr_o18 = mk("o1_8", (P, DC, CUR), FP8, "left")

        # load GRU1 inputs early (SP queue; needed in phase 4)
        inpT_f, fr_inpf = mk("inpT_f", (P, DC, CUR), F32, "left")
        inp_8, fr_inp8 = mk("inp_8", (P, DC, CUR), FP8, "left")
        nc.sync.dma_start(out=inpT_f, in_=inpT_d[:].rearrange("(kc p) t -> p kc t", p=P))
        for n in range(DC):
            VE(n).tensor_copy(inp_8[:, n, :], inpT_f[:, n, :])

        # ================= Phase 3: attention =================
        avT, fr_avT = mk("avT", (P, DC, CUR), FP8, "left")
        with ExitStack() as ph:
            ppp = ph.enter_context(tc.tile_pool(name="ppp", bufs=1, space="PSUM"))
            cpp = ph.enter_context(tc.tile_pool(name="cpp", bufs=2, space="PSUM"))
            ptp = ph.enter_context(tc.tile_pool(name="ptp", bufs=1, space="PSUM"))
            avp = ph.enter_context(tc.tile_pool(name="avp", bufs=1, space="PSUM"))
            pbw = ph.enter_context(tc.tile_pool(name="pbw", bufs=3, side="left"))
            shw = ph.enter_context(tc.tile_pool(name="shw", bufs=5, side="left"))
            smw = ph.enter_context(tc.tile_pool(name="smw", bufs=3, side="left"))
            esw = ph.enter_context(tc.tile_pool(name="esw", bufs=3, side="left"))
            enw = ph.enter_context(tc.tile_pool(name="enw", bufs=2, side="left"))
            atw = ph.enter_context(tc.tile_pool(name="atw", bufs=2, side="left"))
            dnw = ph.enter_context(tc.tile_pool(name="dnw", bufs=2, side="left"))
            scnt = 0

            def head_scores(h):
                """Scores + shift round trip + softmax numerators for head h."""
                nonlocal scnt
                ch, rb = h // 2, (h % 2) * HEAD_DIM
                quh = quT[rb:rb + HEAD_DIM, ch, :]
                qvh = qvT[rb:rb + HEAD_DIM, ch, :]
                kh = kT[rb:rb + HEAD_DIM, ch, :]
                rh = rT[rb:rb + HEAD_DIM, ch, :]
                esn = enw.tile([P, TCC, FULL], FP8, name="esn")
                den = dnw.tile([P, TCC], F32, name="den")
                rec = dnw.tile([P, TCC], F32, name="rec")
                shps = []
                for ic in range(TCC):
                    s_t = scr[scnt % n_scr]
                    scnt += 1
                    c0 = 384 - 128 * ic          # first rel col needed
                    wp = 1024 - c0               # pos width
                    wr = (ic + 5) * 128          # shifted-read width
                    pp = ppp.tile([P, 1024], F32, name="pp", tag="pp")
                    nc.tensor.matmul(pp[:, c0:512], lhsT=qvh[:, ic * P:(ic + 1) * P],
                                     rhs=rh[:, c0:512], start=True, stop=True)
                    nc.tensor.matmul(pp[:, 512:1024], lhsT=qvh[:, ic * P:(ic + 1) * P],
                                     rhs=rh[:, 512:1024], start=True, stop=True)
                    pb = pbw.tile([P, 1024], BF16, name="pb")
                    nc.scalar.copy(pb[:, 0:wp], pp[:, c0:1024])
                    nc.sync.dma_start(out=s_t[:, c0:1024], in_=pb[:, 0:wp])
                    shp = shw.tile([P, FULL], BF16, name="shp")
                    shift_ap = bass.AP(tensor=s_t.tensor,
                                       offset=s_t.offset + 511 - 128 * ic,
                                       ap=[[1535, P], [1, wr]])
                    nc.sync.dma_start(out=shp[:, 0:wr], in_=shift_ap)
                    shps.append(shp)
                for ic in range(TCC):
                    wr = (ic + 5) * 128
                    shp = shps[ic]
                    cp = cpp.tile([P, 1024], F32, name="cp", tag="cp")
                    nc.tensor.matmul(cp[:, 0:512], lhsT=quh[:, ic * P:(ic + 1) * P],
                                     rhs=kh[:, 0:512], start=True, stop=True)
                    nc.tensor.matmul(cp[:, 512:wr], lhsT=quh[:, ic * P:(ic + 1) * P],
                                     rhs=kh[:, 512:wr], start=True, stop=True)
                    sm = smw.tile([P, FULL], BF16, name="sm")
                    VE(h + ic).tensor_add(sm[:, 0:wr], cp[:, 0:wr], shp[:, 0:wr])
                    es = esw.tile([P, FULL], BF16, name="es")
                    nc.scalar.activation(out=es[:, 0:wr], in_=sm[:, 0:wr],
                                         func=Act.Exp, scale=SCALE,
                                         accum_out=den[:, ic:ic + 1])
                    nc.vector.reciprocal(out=rec[:, ic:ic + 1],
                                         in_=den[:, ic:ic + 1])
                    VE(h + ic + 1).tensor_scalar(
                        out=esn[:, ic, 0:wr], in0=es[:, 0:wr],
                        scalar1=rec[:, ic:ic + 1], scalar2=ES_S,
                        op0=AluOp.mult, op1=AluOp.mult)
                return esn

            def head_tail(h, esn):
                """Transpose + AV for head h (runs one head behind)."""
                ch, rb = h // 2, (h % 2) * HEAD_DIM
                attnT = atw.tile([P, TCF, 512], FP8, name="attnT")
                nc.gpsimd.memset(attnT[:, 5, 0:128], 0.0)
                nc.gpsimd.memset(attnT[:, 7, 256:384], 0.0)
                for jc in range(TCF):
                    ic0 = max(0, jc - 4)
                    pt = ptp.tile([P, 512], FP8, name="pt", tag="pt")
                    for ic in range(ic0, TCC):
                        nc.tensor.transpose(pt[:, ic * P:(ic + 1) * P],
                                            esn[:, ic, jc * P:(jc + 1) * P],
                                            ident_8)
                    VE(h + jc).tensor_copy(attnT[:, jc, ic0 * P:512],
                                           pt[:, ic0 * P:512])
                av = avp.tile([P, 512], F32, name="av", tag="av")
                for qh in range(2):
                    pairs = [0, 1, 2] if qh == 0 else [0, 1, 2, 3]
                    for i, pr in enumerate(pairs):
                        nc.tensor.matmul(
                            av[0:HEAD_DIM, qh * 256:qh * 256 + 256],
                            lhsT=v_nat[:, 2 * pr:2 * pr + 2,
                                       h * HEAD_DIM:(h + 1) * HEAD_DIM],
                            rhs=attnT[:, 2 * pr:2 * pr + 2,
                                      qh * 256:qh * 256 + 256],
                            perf_mode=DR,
                            start=(i == 0), stop=(i == len(pairs) - 1))
                nc.scalar.mul(avT[rb:rb + HEAD_DIM, ch, :],
                              av[0:HEAD_DIM, :], IES)

            prev = None
            for h in range(HEAD_NUM):
                esn = head_scores(h)
                if prev is not None:
                    head_tail(prev[0], prev[1])
                prev = (h, esn)
            head_tail(prev[0], prev[1])
        fr_qvT(); fr_quT(); fr_rT(); fr_v(); fr_kT()

        # ================= Phase 4: proj + GRU1 =================
        psum_box["p"] = root.enter_context(
            tc.tile_pool(name="psum_d", bufs=4, space="PSUM"))
        psum_box["s"] = root.enter_context(
            tc.tile_pool(name="psum_sd", bufs=2, space="PSUM"))
        a1T, fr_a1T = mk("a1T", (P, DC, CUR), FP8, "right")
        for n in range(DC):
            ps = PS()
            _dr_quads(nc, ps, [(wproj, avT, n * P, 0, KP)])
            nc.scalar.activation(out=a1T[:, n, :], in_=ps, func=Act.Relu,
                                 scale=ISW, bias=bproj_sb[:, n:n + 1])
        fr_avT()

        with ExitStack() as ph:
            _gru(nc, tc, ph, PS, gw_d, 1, a1T, inp_8, inpT_f, nbg1_sb,
                 o1T_f, o1_8, VE)
        fr_inp8(); fr_inpf(); fr_a1T()

        # ================= Phase 5: LN2 =================
        x2T, fr_x2T = mk("x2T", (P, DC, CUR), FP8, "right")
        with ExitStack() as ph:
            lw = ph.enter_context(tc.tile_pool(name="lw", bufs=2, side="left"))
            sqp = ph.enter_context(tc.tile_pool(name="sqp", bufs=1, side="left"))
            sq = sqp.tile([P, DC, 512], FP8, name="sq")
            for n in range(DC):
                VE(n).tensor_mul(sq[:, n, :], o1_8[:, n, :], o1_8[:, n, :])
            s1 = SM()
            for n in range(DC):
                nc.tensor.matmul(s1, lhsT=ones_red8, rhs=o1_8[:, n, :],
                                 start=(n == 0), stop=(n == DC - 1))
            mean = lw.tile([1, 512], F32, name="mean")
            nc.vector.tensor_scalar_mul(mean, s1, 1.0 / D)
            s2 = SM()
            for n in range(DC):
                nc.tensor.matmul(s2, lhsT=ones_red8, rhs=sq[:, n, :],
                                 start=(n == 0), stop=(n == DC - 1))
            m2m = lw.tile([1, 512], F32, name="m2m")
            nc.vector.tensor_scalar_mul(m2m, s2, 1.0 / D)
            var = lw.tile([1, 512], F32, name="var")
            nc.vector.scalar_tensor_tensor(out=var, in0=mean, scalar=1.0,
                                           in1=mean, op0=AluOp.mult,
                                           op1=AluOp.mult)
            nc.vector.tensor_sub(var, m2m, var)
            sd = lw.tile([1, 512], F32, name="sd2")
            nc.scalar.activation(out=sd, in_=var, func=Act.Sqrt,
                                 bias=eps_t[0:1, :])
            rstd = lw.tile([1, 512], F32, name="rstd2")
            nc.vector.reciprocal(out=rstd, in_=sd)
            meanB = lw.tile([P, 512], F32, name="meanB")
            nc.gpsimd.partition_broadcast(meanB, mean)
            rstdB = lw.tile([P, 512], F32, name="rstdB")
            nc.gpsimd.partition_broadcast(rstdB, rstd)
            for n in range(DC):
                t1 = lw.tile([P, 512], F32, name="t1")
                VE(n).tensor_sub(t1, o1T_f[:, n, :], meanB)
                VE(n).tensor_mul(t1, t1, rstdB)
                VE(n + 1).tensor_scalar(out=x2T[:, n, :], in0=t1,
                                        scalar1=ln2g_sb[:, n:n + 1],
                                        scalar2=ln2b_sb[:, n:n + 1],
                                        op0=AluOp.mult, op1=AluOp.add)

        # ================= Phase 6: MLP =================
        with ExitStack() as ph6:
            m1w = ph6.enter_context(tc.tile_pool(name="m1w", bufs=1, side="right"))
            m1T = m1w.tile([P, HC, 512], FP8)
            with ExitStack() as ph:
                w1p = ph.enter_context(tc.tile_pool(name="w1p", bufs=4, side="right"))
                w1r = w1_d[:].rearrange("(kc p) n -> p kc n", p=P)
                for n in range(HC):
                    w1t = w1p.tile([P, DC, P], FP8, name="w1t", tag="w1t")
                    nc.scalar.dma_start(out=w1t, in_=w1r[:, :, n * P:(n + 1) * P])
                    ps = PS()
                    _dr_quads(nc, ps, [(w1t, x2T, 0, 0, KP)])
                    nc.scalar.activation(out=m1T[:, n, :], in_=ps, func=Act.Relu,
                                         scale=ISW, bias=b1_sb[:, n:n + 1])
            m2T, fr_m2T = mk("m2T", (P, DC, CUR), FP8, "left")
            w2p = ph6.enter_context(tc.tile_pool(name="w2p", bufs=3, side="left"))
            w2r = w2_d[:].rearrange("(kc p) n -> p kc n", p=P)
            KPH = list(range(HC // 2))
            for n in range(DC):
                w2t = w2p.tile([P, HC, P], FP8, name="w2t", tag="w2t")
                nc.scalar.dma_start(out=w2t, in_=w2r[:, :, n * P:(n + 1) * P])
                ps = PS()
                _dr_quads(nc, ps, [(w2t, m1T, 0, 0, KPH)])
                nc.scalar.activation(out=m2T[:, n, :], in_=ps, func=Act.Relu,
                                     scale=ISW, bias=b2_sb[:, n:n + 1])
        fr_x2T()

        # ================= Phase 7: GRU2 + output =================
        o2T_f, fr_o2 = mk("o2T_f", (P, DC, CUR), F32, "right")
        with ExitStack() as ph:
            _gru(nc, tc, ph, PS, gw_d, 2, m2T, o1_8, o1T_f, nbg2_sb,
                 o2T_f, None, VE)
        fr_m2T(); fr_o18(); fr_o1f()

        o2r = out_d[:].rearrange("(kc p) t -> p kc t", p=P)
        nc.sync.dma_start(out=o2r, in_=o2T_f[:, :, :])
        fr_o2()


def _gru(nc, tc, ph, PS, gw_d, g, yT, x8, xf, nbg_sb, oT_f, o_8, VE):
    gwp = ph.enter_context(tc.tile_pool(name=f"gw{g}", bufs=3, side="left"))
    gtmp = ph.enter_context(tc.tile_pool(name=f"gt{g}", bufs=3, side="left"))
    gper = ph.enter_context(tc.tile_pool(name=f"gp{g}", bufs=1, side="left"))
    KP = [0, 1, 2, 3]

    def loadw(m):
        w = gwp.tile([P, DC, D], FP8, name=f"gwt_{m}", tag="gwt")
        nc.scalar.dma_start(out=w, in_=gw_d[(g, m)][:].rearrange("(kc p) n -> p kc n", p=P))
        return w

    wr, ur = loadw("Wr"), loadw("Ur")
    rx = gper.tile([P, DC, 512], FP8, name="rx")
    for n in range(DC):
        ps = PS()
        _dr_quads(nc, ps, [(wr, yT, n * P, 0, KP), (ur, x8, n * P, 0, KP)])
        rr = gtmp.tile([P, 512], F32, name="rr")
        nc.scalar.activation(out=rr, in_=ps, func=Act.Sigmoid, scale=ISW)
        VE(n).tensor_mul(rx[:, n, :], rr, xf[:, n, :])
    wz, uz = loadw("Wz"), loadw("Uz")
    zt = gper.tile([P, DC, 512], BF16, name="zt")
    for n in range(DC):
        ps = PS()
        _dr_quads(nc, ps, [(wz, yT, n * P, 0, KP), (uz, x8, n * P, 0, KP)])
        nc.scalar.activation(out=zt[:, n, :], in_=ps, func=Act.Sigmoid,
                             scale=ISW, bias=nbg_sb[:, n:n + 1])
    wg, ug = loadw("Wg"), loadw("Ug")
    for n in range(DC):
        ps = PS()
        _dr_quads(nc, ps, [(wg, yT, n * P, 0, KP), (ug, rx, n * P, 0, KP)])
        ht = gtmp.tile([P, 512], F32, name="ht")
        nc.scalar.activation(out=ht, in_=ps, func=Act.Tanh, scale=ISW)
        VE(n).tensor_sub(ht, ht, xf[:, n, :])
        VE(n + 1).tensor_mul(ht, ht, zt[:, n, :])
        VE(n).tensor_add(oT_f[:, n, :], ht, xf[:, n, :])
        if o_8 is not None:
            VE(n + 1).tensor_copy(o_8[:, n, :], oT_f[:, n, :])


_NC_CACHE = {}


def _get_nc():
    if "nc" not in _NC_CACHE:
        _NC_CACHE["nc"] = _build()
    return _NC_CACHE["nc"]


def _chunk_t(vec):
    n = vec.shape[0] // P
    return np.ascontiguousarray(vec.reshape(n, P).T.astype(np.float32))


def _fp8w(w):
    f8 = ml_dtypes.float8_e4m3
    return np.clip(np.asarray(w, np.float32) * WS, -240.0, 240.0).astype(f8)


def _prep(inputs):
    f32 = np.float32
    f8 = ml_dtypes.float8_e4m3
    inp = np.asarray(inputs["inputs"], f32)
    mem = np.asarray(inputs["memory"], f32)
    pos = np.asarray(inputs["pos_embedding"], f32)[:, 0, :]
    wkv = np.asarray(inputs["Wkv"], f32)

    shared = {
        "posT8": np.clip(np.ascontiguousarray(pos.T), -240, 240).astype(f8),
        "u_t": _chunk_t(np.asarray(inputs["u"], f32).reshape(-1)),
        "v_t": _chunk_t(np.asarray(inputs["v"], f32).reshape(-1)),
        "ln1_g": np.asarray(inputs["ln1_g"], f32),
        "ln1_b": np.asarray(inputs["ln1_b"], f32),
        "ln2_g_t": _chunk_t(np.asarray(inputs["ln2_g"], f32)),
        "ln2_b_t": _chunk_t(np.asarray(inputs["ln2_b"], f32)),
        "bkvK_t": _chunk_t(np.asarray(inputs["bkv"], f32)[0:D]),
        "bkvV_row": np.asarray(inputs["bkv"], f32)[D:2 * D].reshape(1, D),
        "bq_t": _chunk_t(np.asarray(inputs["bq"], f32)),
        "bpos_t": _chunk_t(np.asarray(inputs["bpos"], f32)),
        "bproj_t": _chunk_t(np.asarray(inputs["bproj"], f32)),
        "b1_t": _chunk_t(np.asarray(inputs["mlp_b1"], f32)),
        "b2_t": _chunk_t(np.asarray(inputs["mlp_b2"], f32)),
        "nbg1_t": _chunk_t(-np.asarray(inputs["g1_bg"], f32)),
        "nbg2_t": _chunk_t(-np.asarray(inputs["g2_bg"], f32)),
        "WkvK8": _fp8w(wkv[:, 0:D]),
        "WkvV8": _fp8w(wkv[:, D:2 * D]),
        "Wq8": _fp8w(inputs["Wq"]),
        "Wpos8": _fp8w(inputs["Wpos"]),
        "Wproj8": _fp8w(inputs["Wproj"]),
        "mlp_W18": _fp8w(inputs["mlp_W1"]),
        "mlp_W28": _fp8w(inputs["mlp_W2"]),
    }
    for g in (1, 2):
        for m in ("Wr", "Ur", "Wz", "Uz", "Wg", "Ug"):
            shared[f"g{g}_{m}8"] = _fp8w(inputs[f"g{g}_{m}"])

    in_maps = []
    for b in range(BS):
        im = dict(shared)
        im["x_full"] = np.ascontiguousarray(
            np.concatenate([mem[:, b, :], inp[:, b, :]], axis=0))
        im["inpT"] = np.ascontiguousarray(inp[:, b, :].T)
        in_maps.append(im)
    return in_maps


def _post(out_t):
    """Device output is [D, CUR]; transpose to [CUR, D]."""
    return np.ascontiguousarray(np.asarray(out_t).T.astype(np.float32))


def kernel(**inputs):
    nc = _get_nc()
    in_maps = _prep(inputs)
    res = run_bass_kernel_spmd(nc, in_maps, core_ids=list(range(BS)))
    out = np.stack([_post(res.results[b]["out"]) for b in range(BS)], axis=1)
    return np.ascontiguousarray(out.astype(np.float32))


if __name__ == "__main__":
    _get_nc()
    print("build+compile OK")


# revision 17
# speedup vs baseline: 1.9614x; 1.1137x over previous
"""GTrXL layer (TransformerXL attention + GRU gating) on 8 TRN2 NeuronCores.

Sharding: pure data-parallel over batch (BS=8 -> 1 batch element per core).
No collectives. Per-core Bass/Tile kernel computes the full layer for its
batch element.

Layout convention on-chip: activations are kept TRANSPOSED [feature, token]
(feature on partitions, 128-chunks).

Precision strategy: all big matmuls run in fp8-e4m3 with DoubleRow perf mode
(2 contraction k-tiles per pass -> 2x bf16 throughput). Weights are scaled by
256 on the host before fp8 quantization (keeps values out of the subnormal
range); every PSUM consumer applies a 2^-8 scale. Activations quantize to fp8
at natural scale; softmax weights are scaled by 128 (fp8 S=7) before the AV
matmul and the output rescaled by 2^-7. Elementwise math (LN, GRU combine)
stays f32; logits/es stay bf16.

Relative-shift: pos scores P[i, rel] are computed only for the needed rel
range [384-128*ic, 1024) per 128-query chunk, written to a DRAM scratch of
row stride 1536 whose tail 512 columns are pre-filled with -1e30. The shifted
read  shifted[i, j] = P[i, 511 - 128*ic + j - i]  is a single strided DMA
(row step 1535, per-chunk offset 511-128*ic — this is the CORRECT global
TrXL shift), and the pad lands exactly on the masked region j > i + 512 + 128*ic.

Softmax denominators come free from the exp instruction's accum_out; the
reciprocal is folded into the es -> fp8 normalization (pre-transpose).
"""

import sys

if '/opt/trn_rl_repo' not in sys.path:
    sys.path.insert(0, '/opt/trn_rl_repo')

import numpy as np
import ml_dtypes

import concourse.bass as bass
import concourse.tile as tile
from concourse import bacc, mybir
from concourse.bass_utils import run_bass_kernel_spmd
from concourse.masks import make_identity

BF16 = mybir.dt.bfloat16
F32 = mybir.dt.float32
FP8 = mybir.dt.float8e4

HEAD_NUM, HEAD_DIM = 16, 64
D, HID = 1024, 4096
CUR, PREV, BS = 512, 512, 8
FULL = CUR + PREV
EPS = 1e-5
SCALE = 1.0 / (HEAD_DIM ** 0.5)
P = 128
DC = D // P          # 8 feature chunks
HC = HID // P        # 32 hidden chunks
TCF = FULL // P      # 8 full-token chunks
TCC = CUR // P       # 4 query-token chunks
NEG = -1.0e30
WS = 256.0           # host-side weight scale before fp8 quantization
ISW = 1.0 / WS       # psum de-scale
ES_S = 128.0         # softmax-weight fp8 scale
IES = 1.0 / ES_S

AluOp = mybir.AluOpType
Act = mybir.ActivationFunctionType
DR = mybir.MatmulPerfMode.DoubleRow


def _dram_in(dram, name, shape, dtype):
    return dram.tile(list(shape), dtype, kind="ExternalInput", name=name,
                     uniquify=False)


def _dr_quads(nc, ps, segs):
    """Fill psum [128, 512] via DoubleRow quadrant chains.

    segs: list of (w, x, c0, t0, kpairs) — accumulate over all segs:
      ps[n, t] += sum_k w[k, c0+n] * x[k, t0+t]   (k over kpairs*256 lanes)
    w, x are [P, 2*kpairs.., *] fp8 tiles (chunk dim second).
    """
    for nh in range(2):
        for qh in range(2):
            out = ps[nh * 64:nh * 64 + 64, qh * 256:qh * 256 + 256]
            total = sum(len(s[4]) for s in segs)
            i = 0
            for (w, x, c0, t0, kpairs) in segs:
                for m in kpairs:
                    nc.tensor.matmul(
                        out,
                        lhsT=w[:, 2 * m:2 * m + 2,
                               c0 + nh * 64:c0 + nh * 64 + 64],
                        rhs=x[:, 2 * m:2 * m + 2,
                              t0 + qh * 256:t0 + qh * 256 + 256],
                        perf_mode=DR,
                        start=(i == 0), stop=(i == total - 1))
                    i += 1


def _build():
    nc = bacc.Bacc("TRN2", target_bir_lowering=False)
    with tile.TileContext(nc) as tc:
        _emit(nc, tc)
    nc.compile()
    return nc


def _emit(nc, tc):
    from contextlib import ExitStack

    with ExitStack() as root:
        dram = root.enter_context(tc.tile_pool(name="io", bufs=1, space="DRAM"))

        # ---------------- DRAM I/O ----------------
        x_full = _dram_in(dram, "x_full", (FULL, D), F32)
        inpT_d = _dram_in(dram, "inpT", (D, CUR), F32)
        posT_d = _dram_in(dram, "posT8", (D, FULL), FP8)
        u_d = _dram_in(dram, "u_t", (P, DC), F32)
        v_d = _dram_in(dram, "v_t", (P, DC), F32)
        ln1g_d = _dram_in(dram, "ln1_g", (D,), F32)
        ln1b_d = _dram_in(dram, "ln1_b", (D,), F32)
        ln2g_d = _dram_in(dram, "ln2_g_t", (P, DC), F32)
        ln2b_d = _dram_in(dram, "ln2_b_t", (P, DC), F32)
        bkvK_d = _dram_in(dram, "bkvK_t", (P, DC), F32)
        bkvV_d = _dram_in(dram, "bkvV_row", (1, D), F32)
        bq_d = _dram_in(dram, "bq_t", (P, DC), F32)
        bpos_d = _dram_in(dram, "bpos_t", (P, DC), F32)
        bproj_d = _dram_in(dram, "bproj_t", (P, DC), F32)
        b1_d = _dram_in(dram, "b1_t", (P, HC), F32)
        b2_d = _dram_in(dram, "b2_t", (P, DC), F32)
        nbg1_d = _dram_in(dram, "nbg1_t", (P, DC), F32)
        nbg2_d = _dram_in(dram, "nbg2_t", (P, DC), F32)

        wkvK_d = _dram_in(dram, "WkvK8", (D, D), FP8)
        wkvV_d = _dram_in(dram, "WkvV8", (D, D), FP8)
        wq_d = _dram_in(dram, "Wq8", (D, D), FP8)
        wpos_d = _dram_in(dram, "Wpos8", (D, D), FP8)
        wproj_d = _dram_in(dram, "Wproj8", (D, D), FP8)
        gw_d = {}
        for g in (1, 2):
            for m in ("Wr", "Ur", "Wz", "Uz", "Wg", "Ug"):
                gw_d[(g, m)] = _dram_in(dram, f"g{g}_{m}8", (D, D), FP8)
        w1_d = _dram_in(dram, "mlp_W18", (D, HID), FP8)
        w2_d = _dram_in(dram, "mlp_W28", (HID, D), FP8)

        # transposed output [D, CUR]; host transposes back
        out_d = dram.tile([D, CUR], F32, kind="ExternalOutput", name="out",
                          uniquify=False)

        n_scr = 8
        scr = [dram.tile([P, 1536], BF16, name=f"scr{s}") for s in range(n_scr)]

        # ---------------- constants ----------------
        const = root.enter_context(tc.tile_pool(name="const", bufs=1))
        ident_f = const.tile([P, P], F32)
        make_identity(nc, ident_f)
        ident_8 = const.tile([P, P], FP8)
        make_identity(nc, ident_8)
        ones_red8 = const.tile([P, 1], FP8)
        nc.vector.memset(ones_red8, 1.0)
        eps_t = const.tile([P, 1], F32)
        nc.vector.memset(eps_t, EPS)

        def cload(name, dref, shape, dtype=F32):
            t = const.tile(list(shape), dtype, name=name)
            nc.sync.dma_start(out=t, in_=dref[:])
            return t

        u_sb = cload("u_sb", u_d, (P, DC))
        v_sb = cload("v_sb", v_d, (P, DC))
        ln2g_sb = cload("ln2g_sb", ln2g_d, (P, DC))
        ln2b_sb = cload("ln2b_sb", ln2b_d, (P, DC))
        bkvK_sb = cload("bkvK_sb", bkvK_d, (P, DC))
        bq_sb = cload("bq_sb", bq_d, (P, DC))
        bpos_sb = cload("bpos_sb", bpos_d, (P, DC))
        bproj_sb = cload("bproj_sb", bproj_d, (P, DC))
        b1_sb = cload("b1_sb", b1_d, (P, HC))
        b2_sb = cload("b2_sb", b2_d, (P, DC))
        nbg1_sb = cload("nbg1_sb", nbg1_d, (P, DC))
        nbg2_sb = cload("nbg2_sb", nbg2_d, (P, DC))
        # V bias broadcast to all partitions (free-dim varying)
        bvV_sb = const.tile([P, D], F32, name="bvV_sb")
        nc.sync.dma_start(out=bvV_sb, in_=bass.AP(
            tensor=bkvV_d.tensor, offset=bkvV_d.offset, ap=[[0, P], [1, D]]))

        padw = const.tile([P, 512], BF16)
        nc.vector.memset(padw, NEG)
        for s in range(n_scr):
            nc.sync.dma_start(out=scr[s][:, 1024:1536], in_=padw)

        # engine rotation for copies / elementwise
        vecs = [nc.vector, nc.gpsimd]

        def VE(i):
            return vecs[i % 2]

        # phase-scoped psum pools (PSUM is only 8 banks; attention needs them)
        psum_box = {}

        def PS():
            return psum_box["p"].tile([P, 512], F32, name="ps", tag="ps")

        def SM():
            return psum_box["s"].tile([1, 512], F32, name="sm", tag="sm")

        def mk(name, shape, dtype, side):
            t, fr = tc.tile(list(shape), dtype, name=name, side=side)
            return t, fr

        x1T8, fr_x1T = mk("x1T8", (P, DC, FULL), FP8, "left")

        # ============ Phase 1+2 fused: rT first (warms PE), then LN1 with
        # ============ V/kT/q matmuls interleaved as token chunks complete
        kT, fr_kT = mk("kT", (P, DC, FULL), FP8, "right")
        v_nat, fr_v = mk("v_nat", (P, TCF, D), FP8, "right")
        rT, fr_rT = mk("rT", (P, DC, FULL), FP8, "right")
        quT, fr_quT = mk("quT", (P, DC, CUR), FP8, "right")
        qvT, fr_qvT = mk("qvT", (P, DC, CUR), FP8, "right")

        KP = [0, 1, 2, 3]  # the 4 k-chunk pairs covering D=1024

        with ExitStack() as ph:
            psum_box["p"] = ph.enter_context(
                tc.tile_pool(name="psum12", bufs=4, space="PSUM"))
            wkvp = ph.enter_context(tc.tile_pool(name="wkvp", bufs=1, side="right"))
            wpos = wkvp.tile([P, DC, D], FP8)
            nc.scalar.dma_start(out=wpos, in_=wpos_d[:].rearrange("(kc p) n -> p kc n", p=P))
            posT_sb = wkvp.tile([P, DC, FULL], FP8)
            nc.scalar.dma_start(out=posT_sb, in_=posT_d[:].rearrange("(kc p) f -> p kc f", p=P))
            wkvK = wkvp.tile([P, DC, D], FP8)
            nc.scalar.dma_start(out=wkvK, in_=wkvK_d[:].rearrange("(kc p) n -> p kc n", p=P))
            wkvV = wkvp.tile([P, DC, D], FP8)
            nc.scalar.dma_start(out=wkvV, in_=wkvV_d[:].rearrange("(kc p) n -> p kc n", p=P))
            wq = wkvp.tile([P, DC, D], FP8)
            nc.scalar.dma_start(out=wq, in_=wq_d[:].rearrange("(kc p) n -> p kc n", p=P))

            # rT only needs pos inputs — keeps the PE busy during LN1
            for n in range(DC):
                for fh in range(2):
                    ps = PS()
                    _dr_quads(nc, ps, [(wpos, posT_sb, n * P, fh * 512, KP)])
                    VE(n + fh).tensor_scalar(
                        out=rT[:, n, fh * 512:(fh + 1) * 512], in0=ps,
                        scalar1=ISW, scalar2=bpos_sb[:, n:n + 1],
                        op0=AluOp.mult, op1=AluOp.add)

            ln1c = ph.enter_context(tc.tile_pool(name="ln1c", bufs=1, side="left"))
            ln1g_sb = ln1c.tile([P, D], F32)
            nc.sync.dma_start(out=ln1g_sb, in_=bass.AP(
                tensor=ln1g_d.tensor, offset=ln1g_d.offset, ap=[[0, P], [1, D]]))
            ln1b_sb = ln1c.tile([P, D], F32)
            nc.sync.dma_start(out=ln1b_sb, in_=bass.AP(
                tensor=ln1b_d.tensor, offset=ln1b_d.offset, ap=[[0, P], [1, D]]))

            xw = ph.enter_context(tc.tile_pool(name="xw", bufs=3, side="left"))
            st = ph.enter_context(tc.tile_pool(name="st", bufs=3, side="left"))
            qw = ph.enter_context(tc.tile_pool(name="qw", bufs=3, side="left"))
            ptp = ph.enter_context(tc.tile_pool(name="ptp", bufs=2, space="PSUM"))
            x_t = x_full[:].rearrange("(tc p) d -> p tc d", p=P)

            def kt_half(th):
                for n in range(DC):
                    ps = PS()
                    _dr_quads(nc, ps, [(wkvK, x1T8, n * P, th * 512, KP)])
                    VE(n + th).tensor_scalar(
                        out=kT[:, n, th * 512:(th + 1) * 512], in0=ps,
                        scalar1=ISW, scalar2=bkvK_sb[:, n:n + 1],
                        op0=AluOp.mult, op1=AluOp.add)

            for tcx in range(TCF):
                xt = xw.tile([P, D], F32, name="xt")
                nc.sync.dma_start(out=xt, in_=x_t[:, tcx, :])
                stats = st.tile([P, 2, 6], F32, name="stats")
                nc.vector.bn_stats(out=stats[:, 0, :], in_=xt[:, 0:512])
                nc.vector.bn_stats(out=stats[:, 1, :], in_=xt[:, 512:1024])
                mv = st.tile([P, 2], F32, name="mv")
                nc.vector.bn_aggr(out=mv, in_=stats)
                sd = st.tile([P, 1], F32, name="sd")
                nc.scalar.activation(out=sd, in_=mv[:, 1:2], func=Act.Sqrt,
                                     bias=eps_t)
                rstd = st.tile([P, 1], F32, name="rstd")
                nc.vector.reciprocal(out=rstd, in_=sd)
                xn = xw.tile([P, D], F32, name="xn")
                nc.vector.tensor_scalar(out=xn, in0=xt, scalar1=mv[:, 0:1],
                                        scalar2=rstd, op0=AluOp.subtract,
                                        op1=AluOp.mult)
                x1n = xw.tile([P, D], F32, name="x1n")
                nc.gpsimd.scalar_tensor_tensor(out=x1n, in0=xn, scalar=1.0,
                                               in1=ln1g_sb, op0=AluOp.mult,
                                               op1=AluOp.mult)
                nc.gpsimd.tensor_add(x1n, x1n, ln1b_sb)
                for dc in range(DC):
                    pt = ptp.tile([P, P], F32, name="pt1", tag="pt1")
                    nc.tensor.transpose(pt, x1n[:, dc * P:(dc + 1) * P], ident_f)
                    VE(dc).tensor_copy(x1T8[:, dc, tcx * P:(tcx + 1) * P], pt)
                # V for this token chunk (contraction over features, all ready)
                for nh in range(2):
                    ps = PS()
                    _dr_quads(nc, ps, [(x1T8, wkvV, tcx * P, nh * 512, KP)])
                    VE(tcx + nh).scalar_tensor_tensor(
                        out=v_nat[:, tcx, nh * 512:(nh + 1) * 512], in0=ps,
                        scalar=ISW, in1=bvV_sb[:, nh * 512:(nh + 1) * 512],
                        op0=AluOp.mult, op1=AluOp.add)
                if tcx == 3:
                    kt_half(0)
                if tcx == 7:
                    kt_half(1)
                    for n in range(DC):
                        ps = PS()
                        _dr_quads(nc, ps, [(wq, x1T8, n * P, CUR, KP)])
                        qn = qw.tile([P, 512], F32, name="qn")
                        nc.vector.tensor_scalar(out=qn, in0=ps, scalar1=ISW,
                                                scalar2=bq_sb[:, n:n + 1],
                                                op0=AluOp.mult, op1=AluOp.add)
                        nc.vector.tensor_scalar_add(quT[:, n, :], qn,
                                                    u_sb[:, n:n + 1])
                        nc.gpsimd.tensor_scalar_add(qvT[:, n, :], qn,
                                                    v_sb[:, n:n + 1])
        fr_x1T()

        # prefetch proj weights (SP queue; ACT queue stays free in attention)
        wprp = root.enter_context(tc.tile_pool(name="wprp", bufs=1, side="left"))
        wproj = wprp.tile([P, DC, D], FP8)
        nc.sync.dma_start(out=wproj, in_=wproj_d[:].rearrange("(kc p) n -> p kc n", p=P))

        # reserve GRU output tiles below the inp tiles (LIFO frees)
        o1T_f, fr_o1f = mk("o1T_f", (P, DC, CUR), F32, "left")
        o1_8, fr_o18 = mk("o1_8", (P, DC, CUR), FP8, "left")

        # load GRU1 inputs early (SP queue; needed in phase 4)
        inpT_f, fr_inpf = mk("inpT_f", (P, DC, CUR), F32, "left")
        inp_8, fr_inp8 = mk("inp_8", (P, DC, CUR), FP8, "left")
        nc.sync.dma_start(out=inpT_f, in_=inpT_d[:].rearrange("(kc p) t -> p kc t", p=P))
        for n in range(DC):
            VE(n).tensor_copy(inp_8[:, n, :], inpT_f[:, n, :])

        # ================= Phase 3: attention =================
        avT, fr_avT = mk("avT", (P, DC, CUR), FP8, "left")
        with ExitStack() as ph:
            ppp = ph.enter_context(tc.tile_pool(name="ppp", bufs=1, space="PSUM"))
            cpp = ph.enter_context(tc.tile_pool(name="cpp", bufs=2, space="PSUM"))
            ptp = ph.enter_context(tc.tile_pool(name="ptp", bufs=1, space="PSUM"))
            avp = ph.enter_context(tc.tile_pool(name="avp", bufs=1, space="PSUM"))
            pbw = ph.enter_context(tc.tile_pool(name="pbw", bufs=3, side="left"))
            shw = ph.enter_context(tc.tile_pool(name="shw", bufs=5, side="left"))
            smw = ph.enter_context(tc.tile_pool(name="smw", bufs=3, side="left"))
            esw = ph.enter_context(tc.tile_pool(name="esw", bufs=3, side="left"))
            enw = ph.enter_context(tc.tile_pool(name="enw", bufs=2, side="left"))
            atw = ph.enter_context(tc.tile_pool(name="atw", bufs=2, side="left"))
            dnw = ph.enter_context(tc.tile_pool(name="dnw", bufs=2, side="left"))
            scnt = 0

            def head_scores(h):
                """Scores + shift round trip + softmax numerators for head h."""
                nonlocal scnt
                ch, rb = h // 2, (h % 2) * HEAD_DIM
                quh = quT[rb:rb + HEAD_DIM, ch, :]
                qvh = qvT[rb:rb + HEAD_DIM, ch, :]
                kh = kT[rb:rb + HEAD_DIM, ch, :]
                rh = rT[rb:rb + HEAD_DIM, ch, :]
                esn = enw.tile([P, TCC, FULL], FP8, name="esn")
                den = dnw.tile([P, TCC], F32, name="den")
                rec = dnw.tile([P, TCC], F32, name="rec")
                shps = []
                for ic in range(TCC):
                    s_t = scr[scnt % n_scr]
                    scnt += 1
                    c0 = 384 - 128 * ic          # first rel col needed
                    wp = 1024 - c0               # pos width
                    wr = (ic + 5) * 128          # shifted-read width
                    pp = ppp.tile([P, 1024], F32, name="pp", tag="pp")
                    nc.tensor.matmul(pp[:, c0:512], lhsT=qvh[:, ic * P:(ic + 1) * P],
                                     rhs=rh[:, c0:512], start=True, stop=True)
                    nc.tensor.matmul(pp[:, 512:1024], lhsT=qvh[:, ic * P:(ic + 1) * P],
                                     rhs=rh[:, 512:1024], start=True, stop=True)
                    pb = pbw.tile([P, 1024], BF16, name="pb")
                    if ic % 2 == 0:
                        nc.scalar.copy(pb[:, 0:wp], pp[:, c0:1024])
                    else:
                        VE(h).tensor_copy(pb[:, 0:wp], pp[:, c0:1024])
                    nc.sync.dma_start(out=s_t[:, c0:1024], in_=pb[:, 0:wp])
                    shp = shw.tile([P, FULL], BF16, name="shp")
                    shift_ap = bass.AP(tensor=s_t.tensor,
                                       offset=s_t.offset + 511 - 128 * ic,
                                       ap=[[1535, P], [1, wr]])
                    nc.sync.dma_start(out=shp[:, 0:wr], in_=shift_ap)
                    shps.append(shp)
                for ic in range(TCC):
                    wr = (ic + 5) * 128
                    shp = shps[ic]
                    cp = cpp.tile([P, 1024], F32, name="cp", tag="cp")
                    nc.tensor.matmul(cp[:, 0:512], lhsT=quh[:, ic * P:(ic + 1) * P],
                                     rhs=kh[:, 0:512], start=True, stop=True)
                    nc.tensor.matmul(cp[:, 512:wr], lhsT=quh[:, ic * P:(ic + 1) * P],
                                     rhs=kh[:, 512:wr], start=True, stop=True)
                    sm = smw.tile([P, FULL], BF16, name="sm")
                    VE(h + ic).tensor_add(sm[:, 0:wr], cp[:, 0:wr], shp[:, 0:wr])
                    es = esw.tile([P, FULL], BF16, name="es")
                    nc.scalar.activation(out=es[:, 0:wr], in_=sm[:, 0:wr],
                                         func=Act.Exp, scale=SCALE,
                                         accum_out=den[:, ic:ic + 1])
                    nc.vector.reciprocal(out=rec[:, ic:ic + 1],
                                         in_=den[:, ic:ic + 1])
                    VE(h + ic + 1).tensor_scalar(
                        out=esn[:, ic, 0:wr], in0=es[:, 0:wr],
                        scalar1=rec[:, ic:ic + 1], scalar2=ES_S,
                        op0=AluOp.mult, op1=AluOp.mult)
                return esn

            def head_tail(h, esn):
                """Transpose + AV for head h (runs one head behind)."""
                ch, rb = h // 2, (h % 2) * HEAD_DIM
                attnT = atw.tile([P, TCF, 512], FP8, name="attnT")
                nc.gpsimd.memset(attnT[:, 5, 0:128], 0.0)
                nc.gpsimd.memset(attnT[:, 7, 256:384], 0.0)
                for jc in range(TCF):
                    ic0 = max(0, jc - 4)
                    pt = ptp.tile([P, 512], FP8, name="pt", tag="pt")
                    for ic in range(ic0, TCC):
                        nc.tensor.transpose(pt[:, ic * P:(ic + 1) * P],
                                            esn[:, ic, jc * P:(jc + 1) * P],
                                            ident_8)
                    VE(h + jc).tensor_copy(attnT[:, jc, ic0 * P:512],
                                           pt[:, ic0 * P:512])
                av = avp.tile([P, 512], F32, name="av", tag="av")
                for qh in range(2):
                    pairs = [0, 1, 2] if qh == 0 else [0, 1, 2, 3]
                    for i, pr in enumerate(pairs):
                        nc.tensor.matmul(
                            av[0:HEAD_DIM, qh * 256:qh * 256 + 256],
                            lhsT=v_nat[:, 2 * pr:2 * pr + 2,
                                       h * HEAD_DIM:(h + 1) * HEAD_DIM],
                            rhs=attnT[:, 2 * pr:2 * pr + 2,
                                      qh * 256:qh * 256 + 256],
                            perf_mode=DR,
                            start=(i == 0), stop=(i == len(pairs) - 1))
                nc.vector.tensor_scalar_mul(avT[rb:rb + HEAD_DIM, ch, :],
                                            av[0:HEAD_DIM, :], IES)

            prev = None
            for h in range(HEAD_NUM):
                esn = head_scores(h)
                if prev is not None:
                    head_tail(prev[0], prev[1])
                prev = (h, esn)
            head_tail(prev[0], prev[1])
        fr_qvT(); fr_quT(); fr_rT(); fr_v(); fr_kT()

        # ================= Phase 4: proj + GRU1 =================
        psum_box["p"] = root.enter_context(
            tc.tile_pool(name="psum_d", bufs=4, space="PSUM"))
        psum_box["s"] = root.enter_context(
            tc.tile_pool(name="psum_sd", bufs=2, space="PSUM"))
        a1T, fr_a1T = mk("a1T", (P, DC, CUR), FP8, "right")
        for n in range(DC):
            ps = PS()
            _dr_quads(nc, ps, [(wproj, avT, n * P, 0, KP)])
            nc.scalar.activation(out=a1T[:, n, :], in_=ps, func=Act.Relu,
                                 scale=ISW, bias=bproj_sb[:, n:n + 1])
        fr_avT()

        with ExitStack() as ph:
            _gru(nc, tc, ph, PS, gw_d, 1, a1T, inp_8, inpT_f, nbg1_sb,
                 o1T_f, o1_8, VE)
        fr_inp8(); fr_inpf(); fr_a1T()

        # ================= Phase 5: LN2 =================
        x2T, fr_x2T = mk("x2T", (P, DC, CUR), FP8, "right")
        with ExitStack() as ph:
            lw = ph.enter_context(tc.tile_pool(name="lw", bufs=2, side="left"))
            sqp = ph.enter_context(tc.tile_pool(name="sqp", bufs=1, side="left"))
            sq = sqp.tile([P, DC, 512], FP8, name="sq")
            for n in range(DC):
                VE(n).tensor_mul(sq[:, n, :], o1_8[:, n, :], o1_8[:, n, :])
            s1 = SM()
            for n in range(DC):
                nc.tensor.matmul(s1, lhsT=ones_red8, rhs=o1_8[:, n, :],
                                 start=(n == 0), stop=(n == DC - 1))
            mean = lw.tile([1, 512], F32, name="mean")
            nc.vector.tensor_scalar_mul(mean, s1, 1.0 / D)
            s2 = SM()
            for n in range(DC):
                nc.tensor.matmul(s2, lhsT=ones_red8, rhs=sq[:, n, :],
                                 start=(n == 0), stop=(n == DC - 1))
            m2m = lw.tile([1, 512], F32, name="m2m")
            nc.vector.tensor_scalar_mul(m2m, s2, 1.0 / D)
            var = lw.tile([1, 512], F32, name="var")
            nc.vector.scalar_tensor_tensor(out=var, in0=mean, scalar=1.0,
                                           in1=mean, op0=AluOp.mult,
                                           op1=AluOp.mult)
            nc.vector.tensor_sub(var, m2m, var)
            sd = lw.tile([1, 512], F32, name="sd2")
            nc.scalar.activation(out=sd, in_=var, func=Act.Sqrt,
                                 bias=eps_t[0:1, :])
            rstd = lw.tile([1, 512], F32, name="rstd2")
            nc.vector.reciprocal(out=rstd, in_=sd)
            meanB = lw.tile([P, 512], F32, name="meanB")
            nc.gpsimd.partition_broadcast(meanB, mean)
            rstdB = lw.tile([P, 512], F32, name="rstdB")
            nc.gpsimd.partition_broadcast(rstdB, rstd)
            for n in range(DC):
                t1 = lw.tile([P, 512], F32, name="t1")
                VE(n).tensor_sub(t1, o1T_f[:, n, :], meanB)
                VE(n).tensor_mul(t1, t1, rstdB)
                VE(n + 1).tensor_scalar(out=x2T[:, n, :], in0=t1,
                                        scalar1=ln2g_sb[:, n:n + 1],
                                        scalar2=ln2b_sb[:, n:n + 1],
                                        op0=AluOp.mult, op1=AluOp.add)

        # ================= Phase 6: MLP =================
        with ExitStack() as ph6:
            m1w = ph6.enter_context(tc.tile_pool(name="m1w", bufs=1, side="right"))
            m1T = m1w.tile([P, HC, 512], FP8)
            with ExitStack() as ph:
                w1p = ph.enter_context(tc.tile_pool(name="w1p", bufs=4, side="right"))
                w1r = w1_d[:].rearrange("(kc p) n -> p kc n", p=P)
                for n in range(HC):
                    w1t = w1p.tile([P, DC, P], FP8, name="w1t", tag="w1t")
                    nc.sync.dma_start(out=w1t, in_=w1r[:, :, n * P:(n + 1) * P])
                    ps = PS()
                    _dr_quads(nc, ps, [(w1t, x2T, 0, 0, KP)])
                    nc.scalar.activation(out=m1T[:, n, :], in_=ps, func=Act.Relu,
                                         scale=ISW, bias=b1_sb[:, n:n + 1])
            m2T, fr_m2T = mk("m2T", (P, DC, CUR), FP8, "left")
            w2p = ph6.enter_context(tc.tile_pool(name="w2p", bufs=3, side="left"))
            w2r = w2_d[:].rearrange("(kc p) n -> p kc n", p=P)
            KPH = list(range(HC // 2))
            for n in range(DC):
                w2t = w2p.tile([P, HC, P], FP8, name="w2t", tag="w2t")
                nc.sync.dma_start(out=w2t, in_=w2r[:, :, n * P:(n + 1) * P])
                ps = PS()
                _dr_quads(nc, ps, [(w2t, m1T, 0, 0, KPH)])
                nc.scalar.activation(out=m2T[:, n, :], in_=ps, func=Act.Relu,
                                     scale=ISW, bias=b2_sb[:, n:n + 1])
        fr_x2T()

        # ================= Phase 7: GRU2 + output =================
        o2T_f, fr_o2 = mk("o2T_f", (P, DC, CUR), F32, "right")
        with ExitStack() as ph:
            _gru(nc, tc, ph, PS, gw_d, 2, m2T, o1_8, o1T_f, nbg2_sb,
                 o2T_f, None, VE)
        fr_m2T(); fr_o18(); fr_o1f()

        o2r = out_d[:].rearrange("(kc p) t -> p kc t", p=P)
        nc.sync.dma_start(out=o2r, in_=o2T_f[:, :, :])
        fr_o2()


def _gru(nc, tc, ph, PS, gw_d, g, yT, x8, xf, nbg_sb, oT_f, o_8, VE):
    gwp = ph.enter_context(tc.tile_pool(name=f"gw{g}", bufs=3, side="left"))
    gtmp = ph.enter_context(tc.tile_pool(name=f"gt{g}", bufs=3, side="left"))
    gper = ph.enter_context(tc.tile_pool(name=f"gp{g}", bufs=1, side="left"))
    KP = [0, 1, 2, 3]

    def loadw(m):
        w = gwp.tile([P, DC, D], FP8, name=f"gwt_{m}", tag="gwt")
        nc.sync.dma_start(out=w, in_=gw_d[(g, m)][:].rearrange("(kc p) n -> p kc n", p=P))
        return w

    wr, ur = loadw("Wr"), loadw("Ur")
    rx = gper.tile([P, DC, 512], FP8, name="rx")
    for n in range(DC):
        ps = PS()
        _dr_quads(nc, ps, [(wr, yT, n * P, 0, KP), (ur, x8, n * P, 0, KP)])
        rr = gtmp.tile([P, 512], F32, name="rr")
        nc.scalar.activation(out=rr, in_=ps, func=Act.Sigmoid, scale=ISW)
        VE(n).tensor_mul(rx[:, n, :], rr, xf[:, n, :])
    wz, uz = loadw("Wz"), loadw("Uz")
    zt = gper.tile([P, DC, 512], BF16, name="zt")
    for n in range(DC):
        ps = PS()
        _dr_quads(nc, ps, [(wz, yT, n * P, 0, KP), (uz, x8, n * P, 0, KP)])
        nc.scalar.activation(out=zt[:, n, :], in_=ps, func=Act.Sigmoid,
                             scale=ISW, bias=nbg_sb[:, n:n + 1])
    wg, ug = loadw("Wg"), loadw("Ug")
    for n in range(DC):
        ps = PS()
        _dr_quads(nc, ps, [(wg, yT, n * P, 0, KP), (ug, rx, n * P, 0, KP)])
        ht = gtmp.tile([P, 512], F32, name="ht")
        nc.scalar.activation(out=ht, in_=ps, func=Act.Tanh, scale=ISW)
        VE(n).tensor_sub(ht, ht, xf[:, n, :])
        VE(n + 1).tensor_mul(ht, ht, zt[:, n, :])
        VE(n).tensor_add(oT_f[:, n, :], ht, xf[:, n, :])
        if o_8 is not None:
            VE(n + 1).tensor_copy(o_8[:, n, :], oT_f[:, n, :])


_NC_CACHE = {}


def _get_nc():
    if "nc" not in _NC_CACHE:
        _NC_CACHE["nc"] = _build()
    return _NC_CACHE["nc"]


def _chunk_t(vec):
    n = vec.shape[0] // P
    return np.ascontiguousarray(vec.reshape(n, P).T.astype(np.float32))


def _fp8w(w):
    f8 = ml_dtypes.float8_e4m3
    return np.clip(np.asarray(w, np.float32) * WS, -240.0, 240.0).astype(f8)


def _prep(inputs):
    f32 = np.float32
    f8 = ml_dtypes.float8_e4m3
    inp = np.asarray(inputs["inputs"], f32)
    mem = np.asarray(inputs["memory"], f32)
    pos = np.asarray(inputs["pos_embedding"], f32)[:, 0, :]
    wkv = np.asarray(inputs["Wkv"], f32)

    shared = {
        "posT8": np.clip(np.ascontiguousarray(pos.T), -240, 240).astype(f8),
        "u_t": _chunk_t(np.asarray(inputs["u"], f32).reshape(-1)),
        "v_t": _chunk_t(np.asarray(inputs["v"], f32).reshape(-1)),
        "ln1_g": np.asarray(inputs["ln1_g"], f32),
        "ln1_b": np.asarray(inputs["ln1_b"], f32),
        "ln2_g_t": _chunk_t(np.asarray(inputs["ln2_g"], f32)),
        "ln2_b_t": _chunk_t(np.asarray(inputs["ln2_b"], f32)),
        "bkvK_t": _chunk_t(np.asarray(inputs["bkv"], f32)[0:D]),
        "bkvV_row": np.asarray(inputs["bkv"], f32)[D:2 * D].reshape(1, D),
        "bq_t": _chunk_t(np.asarray(inputs["bq"], f32)),
        "bpos_t": _chunk_t(np.asarray(inputs["bpos"], f32)),
        "bproj_t": _chunk_t(np.asarray(inputs["bproj"], f32)),
        "b1_t": _chunk_t(np.asarray(inputs["mlp_b1"], f32)),
        "b2_t": _chunk_t(np.asarray(inputs["mlp_b2"], f32)),
        "nbg1_t": _chunk_t(-np.asarray(inputs["g1_bg"], f32)),
        "nbg2_t": _chunk_t(-np.asarray(inputs["g2_bg"], f32)),
        "WkvK8": _fp8w(wkv[:, 0:D]),
        "WkvV8": _fp8w(wkv[:, D:2 * D]),
        "Wq8": _fp8w(inputs["Wq"]),
        "Wpos8": _fp8w(inputs["Wpos"]),
        "Wproj8": _fp8w(inputs["Wproj"]),
        "mlp_W18": _fp8w(inputs["mlp_W1"]),
        "mlp_W28": _fp8w(inputs["mlp_W2"]),
    }
    for g in (1, 2):
        for m in ("Wr", "Ur", "Wz", "Uz", "Wg", "Ug"):
            shared[f"g{g}_{m}8"] = _fp8w(inputs[f"g{g}_{m}"])

    in_maps = []
    for b in range(BS):
        im = dict(shared)
        im["x_full"] = np.ascontiguousarray(
            np.concatenate([mem[:, b, :], inp[:, b, :]], axis=0))
        im["inpT"] = np.ascontiguousarray(inp[:, b, :].T)
        in_maps.append(im)
    return in_maps


def _post(out_t):
    """Device output is [D, CUR]; transpose to [CUR, D]."""
    return np.ascontiguousarray(np.asarray(out_t).T.astype(np.float32))


def kernel(**inputs):
    nc = _get_nc()
    in_maps = _prep(inputs)
    res = run_bass_kernel_spmd(nc, in_maps, core_ids=list(range(BS)))
    out = np.stack([_post(res.results[b]["out"]) for b in range(BS)], axis=1)
    return np.ascontiguousarray(out.astype(np.float32))


if __name__ == "__main__":
    _get_nc()
    print("build+compile OK")


# revision 26
# speedup vs baseline: 2.1372x; 1.0897x over previous
"""GTrXL layer (TransformerXL attention + GRU gating) on 8 TRN2 NeuronCores.

Sharding: pure data-parallel over batch (BS=8 -> 1 batch element per core).
No collectives. Per-core Bass/Tile kernel computes the full layer for its
batch element.

Layout convention on-chip: activations are kept TRANSPOSED [feature, token]
(feature on partitions, 128-chunks).

Precision strategy: all big matmuls run in fp8-e4m3 with DoubleRow perf mode
(2 contraction k-tiles per pass -> 2x bf16 throughput). Weights are scaled by
256 on the host before fp8 quantization (keeps values out of the subnormal
range); every PSUM consumer applies a 2^-8 scale. Activations quantize to fp8
at natural scale; softmax weights are scaled by 128 (fp8 S=7) before the AV
matmul and the output rescaled by 2^-7. Elementwise math (LN, GRU combine)
stays f32; logits/es stay bf16.

Relative-shift: pos scores P[i, rel] are computed only for the needed rel
range [384-128*ic, 1024) per 128-query chunk, written to a DRAM scratch of
row stride 1536 whose tail 512 columns are pre-filled with -1e30. The shifted
read  shifted[i, j] = P[i, 511 - 128*ic + j - i]  is a single strided DMA
(row step 1535, per-chunk offset 511-128*ic — this is the CORRECT global
TrXL shift), and the pad lands exactly on the masked region j > i + 512 + 128*ic.

Softmax denominators come free from the exp instruction's accum_out; the
reciprocal is folded into the es -> fp8 normalization (pre-transpose).
"""

import sys

if '/opt/trn_rl_repo' not in sys.path:
    sys.path.insert(0, '/opt/trn_rl_repo')

import numpy as np
import ml_dtypes

import concourse.bass as bass
import concourse.tile as tile
from concourse import bacc, mybir
from concourse.bass_utils import run_bass_kernel_spmd
from concourse.masks import make_identity

BF16 = mybir.dt.bfloat16
F32 = mybir.dt.float32
FP8 = mybir.dt.float8e4

HEAD_NUM, HEAD_DIM = 16, 64
D, HID = 1024, 4096
CUR, PREV, BS = 512, 512, 8
FULL = CUR + PREV
EPS = 1e-5
SCALE = 1.0 / (HEAD_DIM ** 0.5)
P = 128
DC = D // P          # 8 feature chunks
HC = HID // P        # 32 hidden chunks
TCF = FULL // P      # 8 full-token chunks
TCC = CUR // P       # 4 query-token chunks
NEG = -1.0e30
WS = 256.0           # host-side weight scale before fp8 quantization
ISW = 1.0 / WS       # psum de-scale
ES_S = 128.0         # softmax-weight fp8 scale
IES = 1.0 / ES_S

AluOp = mybir.AluOpType
Act = mybir.ActivationFunctionType
DR = mybir.MatmulPerfMode.DoubleRow


def _dram_in(dram, name, shape, dtype):
    return dram.tile(list(shape), dtype, kind="ExternalInput", name=name,
                     uniquify=False)


def _dr_quads(nc, ps, segs):
    """Fill psum [128, 512] via DoubleRow quadrant chains.

    segs: list of (w, x, c0, t0, kpairs) — accumulate over all segs:
      ps[n, t] += sum_k w[k, c0+n] * x[k, t0+t]   (k over kpairs*256 lanes)
    w, x are [P, 2*kpairs.., *] fp8 tiles (chunk dim second).
    """
    for nh in range(2):
        for qh in range(2):
            out = ps[nh * 64:nh * 64 + 64, qh * 256:qh * 256 + 256]
            total = sum(len(s[4]) for s in segs)
            i = 0
            for (w, x, c0, t0, kpairs) in segs:
                for m in kpairs:
                    nc.tensor.matmul(
                        out,
                        lhsT=w[:, 2 * m:2 * m + 2,
                               c0 + nh * 64:c0 + nh * 64 + 64],
                        rhs=x[:, 2 * m:2 * m + 2,
                              t0 + qh * 256:t0 + qh * 256 + 256],
                        perf_mode=DR,
                        start=(i == 0), stop=(i == total - 1))
                    i += 1


def _build():
    nc = bacc.Bacc("TRN2", target_bir_lowering=False)
    with tile.TileContext(nc) as tc:
        _emit(nc, tc)
    nc.compile()
    return nc


def _emit(nc, tc):
    from contextlib import ExitStack

    with ExitStack() as root:
        dram = root.enter_context(tc.tile_pool(name="io", bufs=1, space="DRAM"))

        # ---------------- DRAM I/O ----------------
        x_full = _dram_in(dram, "x_full", (FULL, D), F32)
        inpT_d = _dram_in(dram, "inpT", (D, CUR), F32)
        posT_d = _dram_in(dram, "posT8", (D, FULL), FP8)
        u_d = _dram_in(dram, "u_t", (P, DC), F32)
        v_d = _dram_in(dram, "v_t", (P, DC), F32)
        ln1g_d = _dram_in(dram, "ln1_g", (D,), F32)
        ln1b_d = _dram_in(dram, "ln1_b", (D,), F32)
        ln2g_d = _dram_in(dram, "ln2_g_t", (P, DC), F32)
        ln2b_d = _dram_in(dram, "ln2_b_t", (P, DC), F32)
        bkvK_d = _dram_in(dram, "bkvK_t", (P, DC), F32)
        bkvV_d = _dram_in(dram, "bkvV_row", (1, D), F32)
        bq_d = _dram_in(dram, "bq_t", (P, DC), F32)
        bpos_d = _dram_in(dram, "bpos_t", (P, DC), F32)
        bproj_d = _dram_in(dram, "bproj_t", (P, DC), F32)
        b1_d = _dram_in(dram, "b1_t", (P, HC), F32)
        b2_d = _dram_in(dram, "b2_t", (P, DC), F32)
        nbg1_d = _dram_in(dram, "nbg1_t", (P, DC), F32)
        nbg2_d = _dram_in(dram, "nbg2_t", (P, DC), F32)

        wkvK_d = _dram_in(dram, "WkvK8", (D, D), FP8)
        wkvV_d = _dram_in(dram, "WkvV8", (D, D), FP8)
        wq_d = _dram_in(dram, "Wq8", (D, D), FP8)
        wpos_d = _dram_in(dram, "Wpos8", (D, D), FP8)
        wproj_d = _dram_in(dram, "Wproj8", (D, D), FP8)
        gw_d = {}
        for g in (1, 2):
            for m in ("Wr", "Ur", "Wz", "Uz", "Wg", "Ug"):
                gw_d[(g, m)] = _dram_in(dram, f"g{g}_{m}8", (D, D), FP8)
        w1_d = _dram_in(dram, "mlp_W18", (D, HID), FP8)
        w2_d = _dram_in(dram, "mlp_W28", (HID, D), FP8)

        # transposed output [D, CUR]; host transposes back
        out_d = dram.tile([D, CUR], F32, kind="ExternalOutput", name="out",
                          uniquify=False)

        # per-head scratch: 4 sub-rows (one per query chunk) of width 1536,
        # fp8, tail 512 cols pre-filled with -240 (acts as -inf after exp)
        n_scr = 4
        scr = [dram.tile([P, TCC, 1536], FP8, name=f"scr{s}")
               for s in range(n_scr)]

        # ---------------- constants ----------------
        const = root.enter_context(tc.tile_pool(name="const", bufs=1))
        ident_f = const.tile([P, P], F32)
        make_identity(nc, ident_f)
        ident_8 = const.tile([P, P], FP8)
        make_identity(nc, ident_8)
        ones_red8 = const.tile([P, 1], FP8)
        nc.vector.memset(ones_red8, 1.0)
        eps_t = const.tile([P, 1], F32)
        nc.vector.memset(eps_t, EPS)

        def cload(name, dref, shape, dtype=F32):
            t = const.tile(list(shape), dtype, name=name)
            nc.sync.dma_start(out=t, in_=dref[:])
            return t

        u_sb = cload("u_sb", u_d, (P, DC))
        v_sb = cload("v_sb", v_d, (P, DC))
        ln2g_sb = cload("ln2g_sb", ln2g_d, (P, DC))
        ln2b_sb = cload("ln2b_sb", ln2b_d, (P, DC))
        bkvK_sb = cload("bkvK_sb", bkvK_d, (P, DC))
        bq_sb = cload("bq_sb", bq_d, (P, DC))
        bpos_sb = cload("bpos_sb", bpos_d, (P, DC))
        bproj_sb = cload("bproj_sb", bproj_d, (P, DC))
        b1_sb = cload("b1_sb", b1_d, (P, HC))
        b2_sb = cload("b2_sb", b2_d, (P, DC))
        nbg1_sb = cload("nbg1_sb", nbg1_d, (P, DC))
        nbg2_sb = cload("nbg2_sb", nbg2_d, (P, DC))
        # V bias broadcast to all partitions (free-dim varying)
        bvV_sb = const.tile([P, D], F32, name="bvV_sb")
        nc.sync.dma_start(out=bvV_sb, in_=bass.AP(
            tensor=bkvV_d.tensor, offset=bkvV_d.offset, ap=[[0, P], [1, D]]))

        padw = const.tile([P, TCC, 512], FP8)
        nc.vector.memset(padw, -240.0)
        for s in range(n_scr):
            nc.sync.dma_start(out=scr[s][:, :, 1024:1536], in_=padw)

        # engine rotation for copies / elementwise
        vecs = [nc.vector, nc.gpsimd]

        def VE(i):
            return vecs[i % 2]

        # phase-scoped psum pools (PSUM is only 8 banks; attention needs them)
        psum_box = {}

        def PS():
            return psum_box["p"].tile([P, 512], F32, name="ps", tag="ps")

        def SM():
            return psum_box["s"].tile([1, 512], F32, name="sm", tag="sm")

        def mk(name, shape, dtype, side):
            t, fr = tc.tile(list(shape), dtype, name=name, side=side)
            return t, fr

        x1T8, fr_x1T = mk("x1T8", (P, DC, FULL), FP8, "left")

        # ============ Phase 1+2 fused: rT first (warms PE), then LN1 with
        # ============ V/kT/q matmuls interleaved as token chunks complete
        kT, fr_kT = mk("kT", (P, DC, FULL), FP8, "right")
        v_nat, fr_v = mk("v_nat", (P, TCF, D), FP8, "right")
        rT, fr_rT = mk("rT", (P, DC, FULL), FP8, "right")
        quT, fr_quT = mk("quT", (P, DC, CUR), FP8, "right")
        qvT, fr_qvT = mk("qvT", (P, DC, CUR), FP8, "right")

        KP = [0, 1, 2, 3]  # the 4 k-chunk pairs covering D=1024

        with ExitStack() as ph:
            psum_box["p"] = ph.enter_context(
                tc.tile_pool(name="psum12", bufs=4, space="PSUM"))
            wkvp = ph.enter_context(tc.tile_pool(name="wkvp", bufs=1, side="right"))
            wpos = wkvp.tile([P, DC, D], FP8)
            nc.scalar.dma_start(out=wpos, in_=wpos_d[:].rearrange("(kc p) n -> p kc n", p=P))
            posT_sb = wkvp.tile([P, DC, FULL], FP8)
            nc.scalar.dma_start(out=posT_sb, in_=posT_d[:].rearrange("(kc p) f -> p kc f", p=P))
            wkvK = wkvp.tile([P, DC, D], FP8)
            nc.scalar.dma_start(out=wkvK, in_=wkvK_d[:].rearrange("(kc p) n -> p kc n", p=P))
            wkvV = wkvp.tile([P, DC, D], FP8)
            nc.scalar.dma_start(out=wkvV, in_=wkvV_d[:].rearrange("(kc p) n -> p kc n", p=P))
            wq = wkvp.tile([P, DC, D], FP8)
            nc.scalar.dma_start(out=wq, in_=wq_d[:].rearrange("(kc p) n -> p kc n", p=P))

            # rT only needs pos inputs — keeps the PE busy during LN1
            for n in range(DC):
                for fh in range(2):
                    ps = PS()
                    _dr_quads(nc, ps, [(wpos, posT_sb, n * P, fh * 512, KP)])
                    VE(n + fh).tensor_scalar(
                        out=rT[:, n, fh * 512:(fh + 1) * 512], in0=ps,
                        scalar1=ISW, scalar2=bpos_sb[:, n:n + 1],
                        op0=AluOp.mult, op1=AluOp.add)

            ln1c = ph.enter_context(tc.tile_pool(name="ln1c", bufs=1, side="left"))
            ln1g_sb = ln1c.tile([P, D], F32)
            nc.sync.dma_start(out=ln1g_sb, in_=bass.AP(
                tensor=ln1g_d.tensor, offset=ln1g_d.offset, ap=[[0, P], [1, D]]))
            ln1b_sb = ln1c.tile([P, D], F32)
            nc.sync.dma_start(out=ln1b_sb, in_=bass.AP(
                tensor=ln1b_d.tensor, offset=ln1b_d.offset, ap=[[0, P], [1, D]]))

            xw = ph.enter_context(tc.tile_pool(name="xw", bufs=3, side="left"))
            st = ph.enter_context(tc.tile_pool(name="st", bufs=3, side="left"))
            qw = ph.enter_context(tc.tile_pool(name="qw", bufs=3, side="left"))
            ptp = ph.enter_context(tc.tile_pool(name="ptp", bufs=2, space="PSUM"))
            x_t = x_full[:].rearrange("(tc p) d -> p tc d", p=P)

            def kt_half(th):
                for n in range(DC):
                    ps = PS()
                    _dr_quads(nc, ps, [(wkvK, x1T8, n * P, th * 512, KP)])
                    VE(n + th).tensor_scalar(
                        out=kT[:, n, th * 512:(th + 1) * 512], in0=ps,
                        scalar1=ISW, scalar2=bkvK_sb[:, n:n + 1],
                        op0=AluOp.mult, op1=AluOp.add)

            for tcx in range(TCF):
                xt = xw.tile([P, D], F32, name="xt")
                nc.sync.dma_start(out=xt, in_=x_t[:, tcx, :])
                stats = st.tile([P, 2, 6], F32, name="stats")
                nc.vector.bn_stats(out=stats[:, 0, :], in_=xt[:, 0:512])
                nc.vector.bn_stats(out=stats[:, 1, :], in_=xt[:, 512:1024])
                mv = st.tile([P, 2], F32, name="mv")
                nc.vector.bn_aggr(out=mv, in_=stats)
                sd = st.tile([P, 1], F32, name="sd")
                nc.scalar.activation(out=sd, in_=mv[:, 1:2], func=Act.Sqrt,
                                     bias=eps_t)
                rstd = st.tile([P, 1], F32, name="rstd")
                nc.vector.reciprocal(out=rstd, in_=sd)
                xn = xw.tile([P, D], F32, name="xn")
                nc.vector.tensor_scalar(out=xn, in0=xt, scalar1=mv[:, 0:1],
                                        scalar2=rstd, op0=AluOp.subtract,
                                        op1=AluOp.mult)
                x1n = xw.tile([P, D], F32, name="x1n")
                nc.gpsimd.scalar_tensor_tensor(out=x1n, in0=xn, scalar=1.0,
                                               in1=ln1g_sb, op0=AluOp.mult,
                                               op1=AluOp.mult)
                nc.gpsimd.tensor_add(x1n, x1n, ln1b_sb)
                for dc in range(DC):
                    pt = ptp.tile([P, P], F32, name="pt1", tag="pt1")
                    nc.tensor.transpose(pt, x1n[:, dc * P:(dc + 1) * P], ident_f)
                    VE(dc).tensor_copy(x1T8[:, dc, tcx * P:(tcx + 1) * P], pt)
                # V for this token chunk (contraction over features, all ready)
                for nh in range(2):
                    ps = PS()
                    _dr_quads(nc, ps, [(x1T8, wkvV, tcx * P, nh * 512, KP)])
                    VE(tcx + nh).scalar_tensor_tensor(
                        out=v_nat[:, tcx, nh * 512:(nh + 1) * 512], in0=ps,
                        scalar=ISW, in1=bvV_sb[:, nh * 512:(nh + 1) * 512],
                        op0=AluOp.mult, op1=AluOp.add)
                if tcx == 3:
                    kt_half(0)
                if tcx == 7:
                    kt_half(1)
                    for n in range(DC):
                        ps = PS()
                        _dr_quads(nc, ps, [(wq, x1T8, n * P, CUR, KP)])
                        qn = qw.tile([P, 512], F32, name="qn")
                        nc.vector.tensor_scalar(out=qn, in0=ps, scalar1=ISW,
                                                scalar2=bq_sb[:, n:n + 1],
                                                op0=AluOp.mult, op1=AluOp.add)
                        nc.vector.tensor_scalar_add(quT[:, n, :], qn,
                                                    u_sb[:, n:n + 1])
                        nc.gpsimd.tensor_scalar_add(qvT[:, n, :], qn,
                                                    v_sb[:, n:n + 1])
        fr_x1T()

        # prefetch proj + GRU1 r-gate weights (SP queue, before attention)
        wprp = root.enter_context(tc.tile_pool(name="wprp", bufs=1, side="left"))
        wproj = wprp.tile([P, DC, D], FP8)
        nc.sync.dma_start(out=wproj, in_=wproj_d[:].rearrange("(kc p) n -> p kc n", p=P))
        wr1 = wprp.tile([P, DC, D], FP8)
        nc.sync.dma_start(out=wr1, in_=gw_d[(1, "Wr")][:].rearrange("(kc p) n -> p kc n", p=P))
        ur1 = wprp.tile([P, DC, D], FP8)
        nc.sync.dma_start(out=ur1, in_=gw_d[(1, "Ur")][:].rearrange("(kc p) n -> p kc n", p=P))

        # reserve GRU output tiles below the inp tiles (LIFO frees)
        o1T_f, fr_o1f = mk("o1T_f", (P, DC, CUR), F32, "left")
        o1_8, fr_o18 = mk("o1_8", (P, DC, CUR), FP8, "left")

        # load GRU1 inputs early (SP queue; needed in phase 4)
        inpT_f, fr_inpf = mk("inpT_f", (P, DC, CUR), F32, "left")
        inp_8, fr_inp8 = mk("inp_8", (P, DC, CUR), FP8, "left")
        nc.sync.dma_start(out=inpT_f, in_=inpT_d[:].rearrange("(kc p) t -> p kc t", p=P))
        for n in range(DC):
            VE(n).tensor_copy(inp_8[:, n, :], inpT_f[:, n, :])

        # ================= Phase 3: attention =================
        avT, fr_avT = mk("avT", (P, DC, CUR), FP8, "left")
        with ExitStack() as ph:
            ppp = ph.enter_context(tc.tile_pool(name="ppp", bufs=1, space="PSUM"))
            cpp = ph.enter_context(tc.tile_pool(name="cpp", bufs=2, space="PSUM"))
            ptp = ph.enter_context(tc.tile_pool(name="ptp", bufs=1, space="PSUM"))
            avp = ph.enter_context(tc.tile_pool(name="avp", bufs=1, space="PSUM"))
            pbw = ph.enter_context(tc.tile_pool(name="pbw", bufs=3, side="left"))
            shw = ph.enter_context(tc.tile_pool(name="shw", bufs=5, side="left"))
            smw = ph.enter_context(tc.tile_pool(name="smw", bufs=3, side="left"))
            esw = ph.enter_context(tc.tile_pool(name="esw", bufs=3, side="left"))
            enw = ph.enter_context(tc.tile_pool(name="enw", bufs=2, side="left"))
            atw = ph.enter_context(tc.tile_pool(name="atw", bufs=2, side="left"))
            dnw = ph.enter_context(tc.tile_pool(name="dnw", bufs=2, side="left"))
            scnt = 0

            def head_scores(h):
                """Scores + shift round trip + softmax numerators for head h."""
                nonlocal scnt
                ch, rb = h // 2, (h % 2) * HEAD_DIM
                quh = quT[rb:rb + HEAD_DIM, ch, :]
                qvh = qvT[rb:rb + HEAD_DIM, ch, :]
                kh = kT[rb:rb + HEAD_DIM, ch, :]
                rh = rT[rb:rb + HEAD_DIM, ch, :]
                esn = enw.tile([P, TCC, FULL], FP8, name="esn")
                den = dnw.tile([P, TCC], F32, name="den")
                rec = dnw.tile([P, TCC], F32, name="rec")
                s_t = scr[scnt % n_scr]
                scnt += 1
                pb = pbw.tile([P, TCC, 1024], FP8, name="pb")
                for ic in range(TCC - 1):
                    nc.gpsimd.memset(pb[:, ic, 0:384 - 128 * ic], 0.0)
                for ic in range(TCC):
                    c0 = 384 - 128 * ic          # first rel col needed
                    pp = ppp.tile([P, 1024], F32, name="pp", tag="pp")
                    nc.tensor.matmul(pp[:, c0:512], lhsT=qvh[:, ic * P:(ic + 1) * P],
                                     rhs=rh[:, c0:512], start=True, stop=True)
                    nc.tensor.matmul(pp[:, 512:1024], lhsT=qvh[:, ic * P:(ic + 1) * P],
                                     rhs=rh[:, 512:1024], start=True, stop=True)
                    if ic % 2 == 0:
                        nc.scalar.copy(pb[:, ic, c0:1024], pp[:, c0:1024])
                    else:
                        VE(h).tensor_copy(pb[:, ic, c0:1024], pp[:, c0:1024])
                # one combined scratch write for all 4 query chunks
                nc.sync.dma_start(out=s_t[:, :, 0:1024], in_=pb)
                shps = []
                for ic in range(TCC):
                    wr = (ic + 5) * 128          # shifted-read width
                    shp = shw.tile([P, FULL], FP8, name="shp")
                    shift_ap = bass.AP(tensor=s_t.tensor,
                                       offset=s_t.offset + 1536 * ic + 511 - 128 * ic,
                                       ap=[[TCC * 1536 - 1, P], [1, wr]])
                    nc.sync.dma_start(out=shp[:, 0:wr], in_=shift_ap)
                    shps.append(shp)
                for ic in range(TCC):
                    wr = (ic + 5) * 128
                    shp = shps[ic]
                    cp = cpp.tile([P, 1024], F32, name="cp", tag="cp")
                    nc.tensor.matmul(cp[:, 0:512], lhsT=quh[:, ic * P:(ic + 1) * P],
                                     rhs=kh[:, 0:512], start=True, stop=True)
                    nc.tensor.matmul(cp[:, 512:wr], lhsT=quh[:, ic * P:(ic + 1) * P],
                                     rhs=kh[:, 512:wr], start=True, stop=True)
                    sm = smw.tile([P, FULL], BF16, name="sm")
                    VE(h + ic).tensor_add(sm[:, 0:wr], cp[:, 0:wr], shp[:, 0:wr])
                    es = esw.tile([P, FULL], BF16, name="es")
                    nc.scalar.activation(out=es[:, 0:wr], in_=sm[:, 0:wr],
                                         func=Act.Exp, scale=SCALE,
                                         accum_out=den[:, ic:ic + 1])
                    nc.vector.reciprocal(out=rec[:, ic:ic + 1],
                                         in_=den[:, ic:ic + 1])
                    VE(h + ic + 1).tensor_scalar(
                        out=esn[:, ic, 0:wr], in0=es[:, 0:wr],
                        scalar1=rec[:, ic:ic + 1], scalar2=ES_S,
                        op0=AluOp.mult, op1=AluOp.mult)
                return esn

            def head_tail(h, esn):
                """Transpose + AV for head h (runs one head behind)."""
                ch, rb = h // 2, (h % 2) * HEAD_DIM
                attnT = atw.tile([P, TCF, 512], FP8, name="attnT")
                nc.gpsimd.memset(attnT[:, 5, 0:128], 0.0)
                nc.gpsimd.memset(attnT[:, 7, 256:384], 0.0)
                for jc in range(TCF):
                    ic0 = max(0, jc - 4)
                    pt = ptp.tile([P, 512], FP8, name="pt", tag="pt")
                    for ic in range(ic0, TCC):
                        nc.tensor.transpose(pt[:, ic * P:(ic + 1) * P],
                                            esn[:, ic, jc * P:(jc + 1) * P],
                                            ident_8)
                    VE(h + jc).tensor_copy(attnT[:, jc, ic0 * P:512],
                                           pt[:, ic0 * P:512])
                av = avp.tile([P, 512], F32, name="av", tag="av")
                for qh in range(2):
                    pairs = [0, 1, 2] if qh == 0 else [0, 1, 2, 3]
                    for i, pr in enumerate(pairs):
                        nc.tensor.matmul(
                            av[0:HEAD_DIM, qh * 256:qh * 256 + 256],
                            lhsT=v_nat[:, 2 * pr:2 * pr + 2,
                                       h * HEAD_DIM:(h + 1) * HEAD_DIM],
                            rhs=attnT[:, 2 * pr:2 * pr + 2,
                                      qh * 256:qh * 256 + 256],
                            perf_mode=DR,
                            start=(i == 0), stop=(i == len(pairs) - 1))
                nc.vector.tensor_scalar_mul(avT[rb:rb + HEAD_DIM, ch, :],
                                            av[0:HEAD_DIM, :], IES)

            prev = None
            for h in range(HEAD_NUM):
                esn = head_scores(h)
                if prev is not None:
                    head_tail(prev[0], prev[1])
                prev = (h, esn)
            head_tail(prev[0], prev[1])
        fr_qvT(); fr_quT(); fr_rT(); fr_v(); fr_kT()

        # ================= Phase 4: proj + GRU1 =================
        psum_box["p"] = root.enter_context(
            tc.tile_pool(name="psum_d", bufs=4, space="PSUM"))
        psum_box["s"] = root.enter_context(
            tc.tile_pool(name="psum_sd", bufs=2, space="PSUM"))
        a1T, fr_a1T = mk("a1T", (P, DC, CUR), FP8, "right")
        for n in range(DC):
            ps = PS()
            _dr_quads(nc, ps, [(wproj, avT, n * P, 0, KP)])
            nc.scalar.activation(out=a1T[:, n, :], in_=ps, func=Act.Relu,
                                 scale=ISW, bias=bproj_sb[:, n:n + 1])
        fr_avT()

        with ExitStack() as ph:
            _gru(nc, tc, ph, PS, gw_d, 1, a1T, inp_8, inpT_f, nbg1_sb,
                 o1T_f, o1_8, VE, pre=(wr1, ur1))
        fr_inp8(); fr_inpf(); fr_a1T()

        # ================= Phase 5: LN2 =================
        x2T, fr_x2T = mk("x2T", (P, DC, CUR), FP8, "right")
        with ExitStack() as ph:
            lw = ph.enter_context(tc.tile_pool(name="lw", bufs=2, side="left"))
            sqp = ph.enter_context(tc.tile_pool(name="sqp", bufs=1, side="left"))
            sq = sqp.tile([P, DC, 512], FP8, name="sq")
            for n in range(DC):
                VE(n).tensor_mul(sq[:, n, :], o1_8[:, n, :], o1_8[:, n, :])
            s1 = SM()
            for n in range(DC):
                nc.tensor.matmul(s1, lhsT=ones_red8, rhs=o1_8[:, n, :],
                                 start=(n == 0), stop=(n == DC - 1))
            mean = lw.tile([1, 512], F32, name="mean")
            nc.vector.tensor_scalar_mul(mean, s1, 1.0 / D)
            s2 = SM()
            for n in range(DC):
                nc.tensor.matmul(s2, lhsT=ones_red8, rhs=sq[:, n, :],
                                 start=(n == 0), stop=(n == DC - 1))
            m2m = lw.tile([1, 512], F32, name="m2m")
            nc.vector.tensor_scalar_mul(m2m, s2, 1.0 / D)
            var = lw.tile([1, 512], F32, name="var")
            nc.vector.scalar_tensor_tensor(out=var, in0=mean, scalar=1.0,
                                           in1=mean, op0=AluOp.mult,
                                           op1=AluOp.mult)
            nc.vector.tensor_sub(var, m2m, var)
            sd = lw.tile([1, 512], F32, name="sd2")
            nc.scalar.activation(out=sd, in_=var, func=Act.Sqrt,
                                 bias=eps_t[0:1, :])
            rstd = lw.tile([1, 512], F32, name="rstd2")
            nc.vector.reciprocal(out=rstd, in_=sd)
            meanB = lw.tile([P, 512], F32, name="meanB")
            nc.gpsimd.partition_broadcast(meanB, mean)
            rstdB = lw.tile([P, 512], F32, name="rstdB")
            nc.gpsimd.partition_broadcast(rstdB, rstd)
            for n in range(DC):
                t1 = lw.tile([P, 512], F32, name="t1")
                VE(n).tensor_sub(t1, o1T_f[:, n, :], meanB)
                VE(n).tensor_mul(t1, t1, rstdB)
                VE(n + 1).tensor_scalar(out=x2T[:, n, :], in0=t1,
                                        scalar1=ln2g_sb[:, n:n + 1],
                                        scalar2=ln2b_sb[:, n:n + 1],
                                        op0=AluOp.mult, op1=AluOp.add)

        # ================= Phase 6: MLP =================
        with ExitStack() as ph6:
            m1w = ph6.enter_context(tc.tile_pool(name="m1w", bufs=1, side="right"))
            m1T = m1w.tile([P, HC, 512], FP8)
            w12p = ph6.enter_context(tc.tile_pool(name="w12p", bufs=1, side="right"))
            w1t = w12p.tile([P, DC, HID], FP8, name="w1t")
            nc.sync.dma_start(out=w1t, in_=w1_d[:].rearrange("(kc p) n -> p kc n", p=P))
            w2t = w12p.tile([P, HC, D], FP8, name="w2t")
            nc.sync.dma_start(out=w2t, in_=w2_d[:].rearrange("(kc p) n -> p kc n", p=P))
            for n in range(HC):
                ps = PS()
                _dr_quads(nc, ps, [(w1t, x2T, n * P, 0, KP)])
                nc.scalar.activation(out=m1T[:, n, :], in_=ps, func=Act.Relu,
                                     scale=ISW, bias=b1_sb[:, n:n + 1])
            m2T, fr_m2T = mk("m2T", (P, DC, CUR), FP8, "left")
            KPH = list(range(HC // 2))
            for n in range(DC):
                ps = PS()
                _dr_quads(nc, ps, [(w2t, m1T, n * P, 0, KPH)])
                nc.scalar.activation(out=m2T[:, n, :], in_=ps, func=Act.Relu,
                                     scale=ISW, bias=b2_sb[:, n:n + 1])
        fr_x2T()

        # ================= Phase 7: GRU2 + output =================
        o2T_f, fr_o2 = mk("o2T_f", (P, DC, CUR), F32, "right")
        o2r = out_d[:].rearrange("(kc p) t -> p kc t", p=P)

        def out_chunk(n):
            nc.sync.dma_start(out=o2r[:, n, :], in_=o2T_f[:, n, :])

        with ExitStack() as ph:
            _gru(nc, tc, ph, PS, gw_d, 2, m2T, o1_8, o1T_f, nbg2_sb,
                 o2T_f, None, VE, on_chunk=out_chunk)
        fr_m2T(); fr_o18(); fr_o1f()
        fr_o2()


def _gru(nc, tc, ph, PS, gw_d, g, yT, x8, xf, nbg_sb, oT_f, o_8, VE,
         pre=None, on_chunk=None):
    gwp = ph.enter_context(tc.tile_pool(name=f"gw{g}", bufs=4, side="left"))
    gtmp = ph.enter_context(tc.tile_pool(name=f"gt{g}", bufs=3, side="left"))
    gper = ph.enter_context(tc.tile_pool(name=f"gp{g}", bufs=1, side="left"))
    KP = [0, 1, 2, 3]

    def loadw(m):
        w = gwp.tile([P, DC, D], FP8, name=f"gwt_{m}", tag="gwt")
        nc.sync.dma_start(out=w, in_=gw_d[(g, m)][:].rearrange("(kc p) n -> p kc n", p=P))
        return w

    wr, ur = pre if pre is not None else (loadw("Wr"), loadw("Ur"))
    wz, uz = loadw("Wz"), loadw("Uz")
    rx = gper.tile([P, DC, 512], FP8, name="rx")
    for n in range(DC):
        ps = PS()
        _dr_quads(nc, ps, [(wr, yT, n * P, 0, KP), (ur, x8, n * P, 0, KP)])
        rr = gtmp.tile([P, 512], F32, name="rr")
        nc.scalar.activation(out=rr, in_=ps, func=Act.Sigmoid, scale=ISW)
        VE(n).tensor_mul(rx[:, n, :], rr, xf[:, n, :])
    wg, ug = loadw("Wg"), loadw("Ug")
    zt = gper.tile([P, DC, 512], BF16, name="zt")
    for n in range(DC):
        ps = PS()
        _dr_quads(nc, ps, [(wz, yT, n * P, 0, KP), (uz, x8, n * P, 0, KP)])
        nc.scalar.activation(out=zt[:, n, :], in_=ps, func=Act.Sigmoid,
                             scale=ISW, bias=nbg_sb[:, n:n + 1])
    for n in range(DC):
        ps = PS()
        _dr_quads(nc, ps, [(wg, yT, n * P, 0, KP), (ug, rx, n * P, 0, KP)])
        ht = gtmp.tile([P, 512], F32, name="ht")
        nc.scalar.activation(out=ht, in_=ps, func=Act.Tanh, scale=ISW)
        VE(n).tensor_sub(ht, ht, xf[:, n, :])
        VE(n + 1).tensor_mul(ht, ht, zt[:, n, :])
        VE(n).tensor_add(oT_f[:, n, :], ht, xf[:, n, :])
        if o_8 is not None:
            VE(n + 1).tensor_copy(o_8[:, n, :], oT_f[:, n, :])
        if on_chunk is not None:
            on_chunk(n)


_NC_CACHE = {}


def _get_nc():
    if "nc" not in _NC_CACHE:
        _NC_CACHE["nc"] = _build()
    return _NC_CACHE["nc"]


def _chunk_t(vec):
    n = vec.shape[0] // P
    return np.ascontiguousarray(vec.reshape(n, P).T.astype(np.float32))


def _fp8w(w):
    f8 = ml_dtypes.float8_e4m3
    return np.clip(np.asarray(w, np.float32) * WS, -240.0, 240.0).astype(f8)


def _prep(inputs):
    f32 = np.float32
    f8 = ml_dtypes.float8_e4m3
    inp = np.asarray(inputs["inputs"], f32)
    mem = np.asarray(inputs["memory"], f32)
    pos = np.asarray(inputs["pos_embedding"], f32)[:, 0, :]
    wkv = np.asarray(inputs["Wkv"], f32)

    shared = {
        "posT8": np.clip(np.ascontiguousarray(pos.T), -240, 240).astype(f8),
        "u_t": _chunk_t(np.asarray(inputs["u"], f32).reshape(-1)),
        "v_t": _chunk_t(np.asarray(inputs["v"], f32).reshape(-1)),
        "ln1_g": np.asarray(inputs["ln1_g"], f32),
        "ln1_b": np.asarray(inputs["ln1_b"], f32),
        "ln2_g_t": _chunk_t(np.asarray(inputs["ln2_g"], f32)),
        "ln2_b_t": _chunk_t(np.asarray(inputs["ln2_b"], f32)),
        "bkvK_t": _chunk_t(np.asarray(inputs["bkv"], f32)[0:D]),
        "bkvV_row": np.asarray(inputs["bkv"], f32)[D:2 * D].reshape(1, D),
        "bq_t": _chunk_t(np.asarray(inputs["bq"], f32)),
        "bpos_t": _chunk_t(np.asarray(inputs["bpos"], f32)),
        "bproj_t": _chunk_t(np.asarray(inputs["bproj"], f32)),
        "b1_t": _chunk_t(np.asarray(inputs["mlp_b1"], f32)),
        "b2_t": _chunk_t(np.asarray(inputs["mlp_b2"], f32)),
        "nbg1_t": _chunk_t(-np.asarray(inputs["g1_bg"], f32)),
        "nbg2_t": _chunk_t(-np.asarray(inputs["g2_bg"], f32)),
        "WkvK8": _fp8w(wkv[:, 0:D]),
        "WkvV8": _fp8w(wkv[:, D:2 * D]),
        "Wq8": _fp8w(inputs["Wq"]),
        "Wpos8": _fp8w(inputs["Wpos"]),
        "Wproj8": _fp8w(inputs["Wproj"]),
        "mlp_W18": _fp8w(inputs["mlp_W1"]),
        "mlp_W28": _fp8w(inputs["mlp_W2"]),
    }
    for g in (1, 2):
        for m in ("Wr", "Ur", "Wz", "Uz", "Wg", "Ug"):
            shared[f"g{g}_{m}8"] = _fp8w(inputs[f"g{g}_{m}"])

    in_maps = []
    for b in range(BS):
        im = dict(shared)
        im["x_full"] = np.ascontiguousarray(
            np.concatenate([mem[:, b, :], inp[:, b, :]], axis=0))
        im["inpT"] = np.ascontiguousarray(inp[:, b, :].T)
        in_maps.append(im)
    return in_maps


def _post(out_t):
    """Device output is [D, CUR]; transpose to [CUR, D]."""
    return np.ascontiguousarray(np.asarray(out_t).T.astype(np.float32))


def kernel(**inputs):
    nc = _get_nc()
    in_maps = _prep(inputs)
    res = run_bass_kernel_spmd(nc, in_maps, core_ids=list(range(BS)))
    out = np.stack([_post(res.results[b]["out"]) for b in range(BS)], axis=1)
    return np.ascontiguousarray(out.astype(np.float32))


if __name__ == "__main__":
    _get_nc()
    print("build+compile OK")
